# revision 1
# baseline (speedup 1.0000x reference)
"""CRvNN forward kernel for 8x Trainium2 NeuronCores (Bass/Tile).

Strategy
--------
Pure data parallelism: batch 32 -> 4 per core; params replicated; no
collectives.  On-device state lives in TRANSPOSED layout (D=256 on partitions
as 2x128 chunks, sequence position i on the free axis, padded 514 -> 516).

Key algebraic insight: the reference's (S2 x S2) neighbor-probability
matrices are first-order linear recurrences.  With a = active*mask:

    (lnp @ x)[i] = a[i-1]*x[i-1] + (1-a[i-1])*(lnp @ x)[i-1]      (forward)
    (rnp @ x)[i] = a[i+1]*x[i+1] + (1-a[i+1])*(rnp @ x)[i+1]      (backward)
    deact[j]     = a[j]*u[j],  u[j] = tp[j+1] + (1-a[j+1])*u[j+1] (backward)

Each is ONE DVE tensor_tensor_scan per 128-partition chunk (the reference's
EPS=1e-9 inside the (1-a+EPS) products perturbs results by <1e-7 --
negligible).  The S^2 matrices are never materialized and no PE transposes
are ever needed: scans chain in transposed layout, which is exactly the
layout conv/w1/w2 want as lhsT/rhs.

Matmuls (conv 1280x256, w1 512x1024, w2 1024x1024, score matvec) run on PE in
fp32r (1 cyc/row, measured rel err ~1.5e-4; fp32 is 4 cyc/row).  fp32r
operands must be written by rounding producers (DVE/ACT ops with f32r out
dtype), which the scans/activations provide for free.  The w2 path can run
bf16 to save SBUF (env CRVNN_W2DT).  LayerNorm over D (= partitions) uses
ones-vector matmul partition reduction; row vectors (tp, active, LN stats)
are (1, 516) tiles; partition-broadcasts bounce through DRAM (DMA cannot
0-stride broadcast from SBUF).

This walrus build supports only ONE sync wait per instruction; a
post-scheduling pass splits multi-wait instructions into single-wait NOP
chains.
"""
import os
import sys
from contextlib import ExitStack

import numpy as np

sys.path.insert(0, "/opt/trn_rl_repo")

import bass_rust
import concourse.bass as bass
import concourse.mybir as mybir
from concourse.tile import TileContext

F32 = mybir.dt.float32
F32R = mybir.dt.float32r
BF16 = mybir.dt.bfloat16
AL = mybir.AluOpType
AF = mybir.ActivationFunctionType

NCORES = 8
NB = 4            # batch per core
D = 256
DC = 2            # D chunks of 128
S2 = 514
SP = 516          # padded sequence length
SPP = SP + 2      # scan-input tiles have leading+trailing zero pad columns
H = 1024
WIN = 5
EPS = 1e-9

SIM = os.environ.get("CRVNN_SIM", "0") == "1"
TRACE = os.environ.get("CRVNN_TRACE", "0") == "1"
MM_DT = os.environ.get("CRVNN_MMDT", "f32r")
W2_DT = os.environ.get("CRVNN_W2DT", "bf16")
GP_LVL = int(os.environ.get("CRVNN_GP", "1"))

NSPLITS = [(0, 512), (512, SP - 512)]

LAST_EXEC_NS = None
LAST_RES = None

_DT = {"f32": F32, "f32r": F32R, "bf16": BF16}


# --------------------------------------------------------------------------
# post-scheduling fixup: split multi-wait instructions into 1-wait NOP chains
# --------------------------------------------------------------------------
def _split_multiwaits(nc):
    counter = [0]

    def mk_nop(engine, wait):
        counter[0] += 1
        n = bass_rust.InstNoOp(name=f"WFIX-{counter[0]}", ins=[], outs=[])
        n.engine = engine
        n.sync_info = bass_rust.SyncInfo(on_wait=[wait], on_update=[])
        return n

    total = 0
    for f in nc.m.functions:
        for bb in f.blocks:
            out = []
            changed = False
            for inst in list(bb.instructions):
                si = inst.sync_info
                waits = list(si.on_wait) if (si is not None and si.on_wait) else []
                if len(waits) > 1:
                    for w in waits[:-1]:
                        out.append(mk_nop(inst.engine, w))
                    inst.sync_info = bass_rust.SyncInfo(
                        on_wait=[waits[-1]],
                        on_update=list(si.on_update) if si.on_update else [])
                    changed = True
                    total += 1
                out.append(inst)
            if changed:
                bb.instructions = out
    return total


def _bcast_ap(drow):
    """DRAM row AP (1, n) -> partition-broadcast AP (128, n)."""
    return bass.AP(tensor=drow.tensor, offset=drow.offset,
                   ap=[[0, 128]] + drow.ap[1:])


def _f32(ap):
    return ap.bitcast(F32) if ap.dtype != F32 else ap


def _build_program(n_steps, flags):
    nc = bass.Bass()
    R = _DT[MM_DT]
    W2R = _DT[W2_DT]

    seqT_in = nc.declare_dram_parameter("seqT", [NB, DC, 128, SP], F32, isOutput=False)
    mask_in = nc.declare_dram_parameter("mask", [NB, SP], F32, isOutput=False)
    selp_in = nc.declare_dram_parameter("selp", [NB, SP], F32, isOutput=False)
    act0_in = nc.declare_dram_parameter("act0", [NB, SP], F32, isOutput=False)
    nact0_in = nc.declare_dram_parameter("nact0", [NB, SP], F32, isOutput=False)
    itW_in = nc.declare_dram_parameter("itW", [D, D], F32, isOutput=False)
    convW_in = nc.declare_dram_parameter("convW", [WIN * D, D], F32, isOutput=False)
    scW_in = nc.declare_dram_parameter("scWc", [128, DC], F32, isOutput=False)
    w1W_in = nc.declare_dram_parameter("w1W", [2 * D, H], F32, isOutput=False)
    w2W_in = nc.declare_dram_parameter("w2W", [H, 4 * D], F32, isOutput=False)
    noc_in = nc.declare_dram_parameter("noc", [128, DC], F32, isOutput=False)
    ymn_in = nc.declare_dram_parameter("ymnc", [128, DC], F32, isOutput=False)
    opt_in = {}
    for nm, shape in [("itbc", [128, DC]), ("convbc", [128, DC]),
                      ("w1bc", [128, 8]), ("w2bc", [128, 8]), ("scbc", [1, 1]),
                      ("lngc", [128, DC]), ("lnbc", [128, DC])]:
        if flags.get(nm):
            opt_in[nm] = nc.declare_dram_parameter(nm, shape, F32, isOutput=False)
    out_dram = nc.declare_dram_parameter("out", [NB, DC, 128, S2], F32, isOutput=True)

    with TileContext(nc) as tc, ExitStack() as ctx:
        wpool = ctx.enter_context(tc.tile_pool(name="wpool", bufs=1))
        state = ctx.enter_context(tc.tile_pool(name="state", bufs=1))
        work = ctx.enter_context(tc.tile_pool(name="work", bufs=1))
        psum = ctx.enter_context(tc.tile_pool(name="psum", bufs=1, space="PSUM"))
        dram = ctx.enter_context(tc.tile_pool(name="dramp", bufs=1, space="DRAM"))

        # ---------------- weights -> SBUF (round to matmul dtype) -----------
        def load_w(name, dram_ap, shape, dt):
            t = wpool.tile(shape, dt, name=name)
            if dt == F32:
                nc.sync.dma_start(out=t, in_=dram_ap)
            else:
                tmp = work.tile(shape, F32, name=f"{name}_ld", tag="interT")
                nc.sync.dma_start(out=tmp, in_=dram_ap)
                nc.vector.tensor_copy(out=t, in_=tmp)
            return t

        convW_t = [load_w(f"convW{k}", convW_in.ap()[k * 128:(k + 1) * 128, :],
                          [128, D], R) for k in range(10)]
        w1W_t = [load_w(f"w1W{k}", w1W_in.ap()[k * 128:(k + 1) * 128, :],
                        [128, H], R) for k in range(4)]
        w2W_t = [load_w(f"w2W{k}", w2W_in.ap()[k * 128:(k + 1) * 128, :],
                        [128, H], W2R) for k in range(8)]
        scW_t = load_w("scWt", scW_in.ap(), [128, DC], R)
        itW_t = [wpool.tile([128, D], F32, name=f"itW{k}") for k in range(2)]
        for k in range(2):
            nc.sync.dma_start(out=itW_t[k], in_=itW_in.ap()[k * 128:(k + 1) * 128, :])

        noc = wpool.tile([128, DC], F32)
        nc.sync.dma_start(out=noc, in_=noc_in.ap())
        ymnc = wpool.tile([128, DC], F32)
        nc.sync.dma_start(out=ymnc, in_=ymn_in.ap())
        ones_f = wpool.tile([128, 1], F32)
        nc.vector.memset(ones_f, 1.0)
        ones_r = wpool.tile([128, 1], R)
        nc.vector.tensor_copy(out=ones_r, in_=ones_f)
        eps_t = wpool.tile([128, 1], F32)
        nc.vector.memset(eps_t, 1e-5)

        def load_opt(nm, shape):
            if nm not in opt_in:
                return None
            t = wpool.tile(shape, F32, name=f"{nm}_t")
            nc.sync.dma_start(out=t, in_=opt_in[nm].ap())
            return t

        itb_t = load_opt("itbc", [128, DC])
        convb_t = load_opt("convbc", [128, DC])
        w1b_t = load_opt("w1bc", [128, 8])
        w2b_t = load_opt("w2bc", [128, 8])
        scb_t = load_opt("scbc", [1, 1])
        lng_t = load_opt("lngc", [128, DC])
        lnb_t = load_opt("lnbc", [128, DC])

        # ---------------- per-batch persistent state ------------------------
        seqT = [state.tile([128, DC, SP], R, name=f"seqT{b}") for b in range(NB)]
        a_row = [state.tile([1, SP], F32, name=f"a_row{b}") for b in range(NB)]
        mask_r = [state.tile([1, SP], F32, name=f"mask_r{b}") for b in range(NB)]
        selp_r = [state.tile([1, SP], F32, name=f"selp_r{b}") for b in range(NB)]
        for b in range(NB):
            nc.sync.dma_start(out=a_row[b], in_=act0_in.ap()[b:b + 1, :])
            nc.sync.dma_start(out=mask_r[b], in_=mask_in.ap()[b:b + 1, :])
            nc.sync.dma_start(out=selp_r[b], in_=selp_in.ap()[b:b + 1, :])

        # DRAM bounce rows for partition-broadcast
        a_d = [dram.tile([1, SP], F32, name=f"a_d{b}") for b in range(NB)]
        na_d = [dram.tile([1, SP], F32, name=f"na_d{b}") for b in range(NB)]
        ltp_d = [dram.tile([1, SP], F32, name=f"ltp_d{b}") for b in range(NB)]
        rA_d = [dram.tile([1, SP], F32, name=f"rA_d{b}") for b in range(NB)]
        rB_d = [dram.tile([1, SP], F32, name=f"rB_d{b}") for b in range(NB)]
        rC_d = [dram.tile([1, SP], F32, name=f"rC_d{b}") for b in range(NB)]

        
        def work_big(name, tag, dtype=F32, bufs=None):
            return work.tile([128, DC, SP], dtype, name=name, tag=tag, bufs=bufs)

        def row(name):
            return work.tile([1, SP], F32, name=name, tag="rowW", bufs=8)

        def tiny(name):
            return work.tile([1, 1], F32, name=name, tag="tinyW", bufs=6)

        def bc_tile(name):
            return work.tile([128, SP], F32, name=name, tag="bcast", bufs=6)

        def bounce_bcast(row_sb, drow, name):
            """row (1,SP) SBUF -> DRAM -> (128,SP) broadcast tile."""
            if row_sb is not None:
                nc.sync.dma_start(out=drow, in_=row_sb)
            t = bc_tile(name)
            nc.sync.dma_start(out=t, in_=_bcast_ap(drow if not isinstance(drow, bass.AP) else drow))
            return t

        def napad_tile(name):
            """(128, SP+1) broadcast tile; data in cols 1..SP.  Col 0 is
            uninitialized -- scans read it only where multiplied by the zero
            initial state."""
            t = work.tile([128, SPP], F32, name=name, tag="nabP", bufs=2)
            nc.vector.memset(t[:, 0:SPP:SPP - 1], 0.0)
            return t

        def recip(out_r, in_r):
            nc.vector.reciprocal(out=out_r, in_=in_r)

        def tt(out, in0, in1, op, gp=False):
            eng = nc.gpsimd if (gp and GP_LVL > 0) else nc.vector
            eng.tensor_tensor(out=out, in0=in0, in1=in1, op=op)

        def mm(psum_ap, lhsT, rhs_chunks, nsl=NSPLITS):
            K = len(lhsT)
            for (o, s) in nsl:
                for k in range(K):
                    nc.tensor.matmul(psum_ap[:, o:o + s], lhsT[k],
                                     rhs_chunks[k][:, o:o + s],
                                     start=(k == 0), stop=(k == K - 1))

        def gelu_act(out, in_, bias):
            b = bias if bias is not None else 0.0
            if SIM:
                x2 = work.tile([out.shape[0], out.shape[-1]], F32, name="gx2",
                               tag="gelu_tmp", bufs=2)
                nc.scalar.activation(out=x2, in_=in_, func=AF.Square, bias=b)
                nc.vector.tensor_scalar(out=x2, in0=x2, scalar1=0.044715,
                                        scalar2=1.0, op0=AL.mult, op1=AL.add)
                u = work.tile([out.shape[0], out.shape[-1]], F32, name="gu",
                              tag="gelu_tmp2")
                if bias is not None:
                    nc.scalar.activation(out=u, in_=in_, func=AF.Identity, bias=b)
                else:
                    nc.scalar.activation(out=u, in_=in_, func=AF.Copy)
                nc.vector.tensor_tensor(out=x2, in0=x2, in1=u, op=AL.mult)
                nc.scalar.activation(out=x2, in_=x2, func=AF.Tanh,
                                     scale=0.7978845608028654)
                nc.vector.tensor_scalar(out=x2, in0=x2, scalar1=1.0,
                                        scalar2=0.5, op0=AL.add, op1=AL.mult)
                nc.vector.tensor_tensor(out=out, in0=x2, in1=u, op=AL.mult)
            else:
                nc.scalar.activation(out=out, in_=in_, func=AF.Gelu_apprx_tanh,
                                     bias=b, scale=1.0)

        def scan_fwd(out_c, nap, datap):
            """out[i] = data[i-1] + na[i-1]*out[i-1]; data pad supplies z0=0."""
            nc.vector.tensor_tensor_scan(
                out=out_c, data0=nap[:, 0:SP], data1=datap[:, 0:SP],
                initial=0.0, op0=AL.mult, op1=AL.add)

        def scan_bwd(out_c, nap, datap):
            nc.vector.tensor_tensor_scan(
                out=out_c[:, ::-1], data0=nap[:, SPP - 1:1:-1],
                data1=datap[:, SPP - 1:1:-1], initial=0.0,
                op0=AL.mult, op1=AL.add)

        # ---------------- LN stat rows (partition-axis over both chunks) ----
        def ln_rows(src_big, rdt):
            """src (128, DC, SP) of dtype rdt -> (rstd, m*rstd) rows (1, SP)."""
            ps_m = psum.tile([1, SP], F32, name="ps_m", tag="psrow", bufs=2)
            ones = ones_r if rdt != F32 else ones_f
            mm(ps_m, [ones, ones], [src_big[:, 0, :], src_big[:, 1, :]])
            sq = [work.tile([128, SP], rdt, name=f"sq{c}", tag="sq", bufs=2)
                  for c in range(DC)]
            for c in range(DC):
                nc.scalar.activation(out=sq[c], in_=src_big[:, c, :],
                                     func=AF.Square, bias=0.0)
            ps_v = psum.tile([1, SP], F32, name="ps_v", tag="psrow", bufs=2)
            mm(ps_v, [ones, ones], [sq[0], sq[1]])
            m_r = row("m_r")
            nc.scalar.activation(out=m_r, in_=ps_m, func=AF.Copy, scale=1.0 / D)
            v_r = row("v_r")
            nc.scalar.activation(out=v_r, in_=ps_v, func=AF.Copy, scale=1.0 / D)
            msq = row("msq")
            nc.vector.tensor_tensor(out=msq, in0=m_r, in1=m_r, op=AL.mult)
            nc.vector.tensor_tensor(out=v_r, in0=v_r, in1=msq, op=AL.subtract)
            nc.scalar.activation(out=v_r, in_=v_r, func=AF.Sqrt, bias=eps_t[0:1, 0:1])
            rstd = row("rstd")
            recip(rstd, v_r)
            mr = row("mr")
            nc.vector.tensor_tensor(out=mr, in0=m_r, in1=rstd, op=AL.mult)
            return rstd, mr

        def apply_ln_gated(dst_big, pre_big, rAB, rBB, rCB, gateB, b):
            """dst = rAB*pre - rBB [*lng +tpm*lnb] + rCB*seq-like source.

            rCB/gateB None => initial transform (dst = (rA*pre - rB) path only).
            """
            for c in range(DC):
                t1 = work.tile([128, SP], F32, name="t1g", tag="gelu_tmp", bufs=2)
                nc.vector.tensor_tensor(out=t1, in0=rAB, in1=pre_big[:, c, :],
                                        op=AL.mult)
                nc.vector.tensor_tensor(out=t1, in0=t1, in1=rBB, op=AL.subtract)
                if lng_t is not None:
                    nc.vector.tensor_scalar(out=t1, in0=t1,
                                            scalar1=lng_t[:, c:c + 1],
                                            scalar2=None, op0=AL.mult)
                    # + (tpm or mask) * lnb
                    nc.vector.scalar_tensor_tensor(
                        out=t1, in0=gateB, scalar=lnb_t[:, c:c + 1], in1=t1,
                        op0=AL.mult, op1=AL.add)
                if rCB is None:
                    nc.vector.tensor_copy(out=dst_big[:, c, :], in_=t1)
                else:
                    t2 = work.tile([128, SP], F32, name="t2g", tag="gelu_tmp2")
                    tt(t2, rCB, _f32(seqT[b][:, c, :]), AL.mult, gp=True)
                    nc.vector.tensor_tensor(out=dst_big[:, c, :], in0=t1, in1=t2,
                                            op=AL.add)

        # ================= initial transform ================================
        pending0 = None
        for b in range(NB):
            sA = work_big(f"sA{b}", tag="axT")
            nc.sync.dma_start(out=sA, in_=seqT_in.ap()[b].rearrange("c p i -> p c i"))
            pre = work_big(f"pre{b}", tag="compT", dtype=R, bufs=3)
            for c in range(DC):
                ps = psum.tile([128, SP], F32, name=f"ps_pre{b}{c}", tag="psmm", bufs=2)
                mm(ps, [itW_t[k][:, c * 128:(c + 1) * 128] for k in range(2)],
                   [sA[:, 0, :], sA[:, 1, :]])
                if itb_t is not None:
                    nc.scalar.activation(out=pre[:, c, :], in_=ps, func=AF.Identity,
                                         bias=itb_t[:, c:c + 1])
                else:
                    nc.scalar.activation(out=pre[:, c, :], in_=ps, func=AF.Copy)
            rstd, mr = ln_rows(pre, R)
            rA = row(f"rA0_{b}")
            nc.vector.tensor_tensor(out=rA, in0=rstd, in1=mask_r[b], op=AL.mult)
            rB = row(f"rB0_{b}")
            nc.vector.tensor_tensor(out=rB, in0=mr, in1=mask_r[b], op=AL.mult)
            rAB = bounce_bcast(rA, rA_d[b], f"rAB0_{b}")
            rBB = bounce_bcast(rB, rB_d[b], f"rBB0_{b}")
            maskB = None
            if lng_t is not None:
                maskB = bounce_bcast(None, mask_in.ap()[b:b + 1, :], f"mB0_{b}")
            if pending0 is not None:
                pb, ppre, pr = pending0
                apply_ln_gated(seqT[pb], ppre, pr[0], pr[1], None, pr[2], pb)
            pending0 = (b, pre, (rAB, rBB, maskB))
        pb, ppre, pr = pending0
        apply_ln_gated(seqT[pb], ppre, pr[0], pr[1], None, pr[2], pb)

        def emit_tail(b, tsc, comp):
                    # ---- phase D: transition prob + active update (rows) ----
                    masked = row(f"msk{b}")
                    nc.vector.tensor_tensor(out=masked, in0=tsc, in1=selp_r[b],
                                            op=AL.mult)
                    mx = tiny(f"mx{b}")
                    nc.vector.tensor_reduce(out=mx, in_=masked,
                                            axis=mybir.AxisListType.X, op=AL.max)
                    negmx = tiny(f"negmx{b}")
                    nc.vector.tensor_scalar(out=negmx, in0=mx, scalar1=0.0,
                                            scalar2=-1.0, op0=AL.max, op1=AL.mult)
                    et = row(f"et{b}")
                    nc.scalar.activation(out=et, in_=tsc, func=AF.Exp, bias=negmx)
                    nc.vector.tensor_tensor(out=et, in0=et, in1=selp_r[b], op=AL.mult)
                    en = tiny(f"en{b}")
                    nc.scalar.activation(out=en, in_=negmx, func=AF.Exp)
                    nc.vector.tensor_scalar(out=en, in0=en, scalar1=EPS, scalar2=None,
                                            op0=AL.add)
                    den = row(f"den{b}")
                    nc.vector.tensor_scalar(out=den, in0=et, scalar1=en, scalar2=None,
                                            op0=AL.add)
                    dei = row(f"dei{b}")
                    recip(dei, den)
                    den = dei
                    tp = row(f"tp{b}")
                    nc.vector.tensor_tensor(out=tp, in0=et, in1=den, op=AL.mult)
                    nc.sync.dma_start(out=ltp_d[b], in_=tp)

                    # deact scan (padded row tiles) + active update
                    nap = work.tile([1, SPP], F32, name=f"nap{b}", tag="rowP", bufs=4)
                    nc.vector.memset(nap[:, 0:SPP:SPP - 1], 0.0)
                    nc.vector.tensor_scalar(out=nap[:, 1:SP + 1], in0=a_row[b],
                                            scalar1=-1.0, scalar2=1.0,
                                            op0=AL.mult, op1=AL.add)
                    tpp = work.tile([1, SPP], F32, name=f"tpp{b}", tag="rowP", bufs=4)
                    nc.vector.memset(tpp[:, 0:SPP:SPP - 1], 0.0)
                    nc.vector.tensor_copy(out=tpp[:, 1:SP + 1], in_=tp)
                    u = row(f"u{b}")
                    nc.vector.tensor_tensor_scan(
                        out=u[:, ::-1], data0=nap[:, SPP - 1:1:-1],
                        data1=tpp[:, SPP - 1:1:-1], initial=0.0,
                        op0=AL.mult, op1=AL.add)
                    nd = row(f"nd{b}")
                    nc.vector.tensor_tensor(out=nd, in0=a_row[b], in1=u, op=AL.mult)
                    nc.vector.tensor_scalar(out=nd, in0=nd, scalar1=-1.0, scalar2=1.0,
                                            op0=AL.mult, op1=AL.add)
                    nc.vector.tensor_tensor(out=nd, in0=a_row[b], in1=nd, op=AL.mult)
                    nc.vector.tensor_scalar(out=nd, in0=nd, scalar1=0.0, scalar2=1.0,
                                            op0=AL.max, op1=AL.min)
                    nc.vector.tensor_tensor(out=a_row[b], in0=nd, in1=mask_r[b],
                                            op=AL.mult)
                    nc.sync.dma_start(out=a_d[b], in_=a_row[b])
                    nar = row(f"nar{b}")
                    nc.vector.tensor_scalar(out=nar, in0=a_row[b], scalar1=-1.0,
                                            scalar2=1.0, op0=AL.mult, op1=AL.add)
                    nc.sync.dma_start(out=na_d[b], in_=nar)

                    # ---- phase F: LN rows + gating rows ----
                    rstd, mr = ln_rows(comp, R)
                    tpm = row(f"tpm{b}")
                    nc.vector.tensor_tensor(out=tpm, in0=tp, in1=mask_r[b], op=AL.mult)
                    rA = row(f"rA{b}")
                    nc.vector.tensor_tensor(out=rA, in0=tpm, in1=rstd, op=AL.mult)
                    rB = row(f"rB{b}")
                    nc.vector.tensor_tensor(out=rB, in0=tpm, in1=mr, op=AL.mult)
                    rC = row(f"rC{b}")
                    nc.vector.tensor_tensor(out=rC, in0=mask_r[b], in1=tpm,
                                            op=AL.subtract)
                    rAB = bounce_bcast(rA, rA_d[b], f"rAB{b}")
                    rBB = bounce_bcast(rB, rB_d[b], f"rBB{b}")
                    rCB = bounce_bcast(rC, rC_d[b], f"rCB{b}")
                    tpmB = None
                    if lng_t is not None:
                        tpm_d = dram.tile([1, SP], F32, name=f"tpm_d{b}")
                        tpmB = bounce_bcast(tpm, tpm_d, f"tpmB{b}")

                    # stash for deferred phase G (applied during next batch's scans)
                    apply_ln_gated(seqT[b], comp, rAB, rBB, rCB, tpmB, b)

        # ================= main steps =======================================
        pending_tail = []
        for s in range(n_steps):
            for b in range(NB):
                # ---- phase A: broadcasts + base ----
                if s == 0:
                    aB = bc_tile(f"aB{b}")
                    nc.sync.dma_start(out=aB,
                                      in_=_bcast_ap(act0_in.ap()[b:b + 1, :]))
                    naB = napad_tile(f"naB{b}")
                    nc.sync.dma_start(out=naB[:, 1:SP + 1],
                                      in_=_bcast_ap(nact0_in.ap()[b:b + 1, :]))
                else:
                    aB = bounce_bcast(None, a_d[b], f"aB{b}")
                    naB = napad_tile(f"naB{b}")
                    nc.sync.dma_start(out=naB[:, 1:SP + 1], in_=_bcast_ap(na_d[b]))
                baseT = work_big(f"baseT{b}", tag="baseT", dtype=R)
                if s == 0:
                    for c in range(DC):
                        nc.vector.tensor_scalar(
                            out=baseT[:, c, :], in0=_f32(seqT[b][:, c, :]),
                            scalar1=noc[:, c:c + 1], scalar2=None, op0=AL.add)
                else:
                    ltpB = bounce_bcast(None, ltp_d[b], f"ltpB{b}")
                    for c in range(DC):
                        nc.vector.scalar_tensor_tensor(
                            out=baseT[:, c, :], in0=ltpB, scalar=ymnc[:, c:c + 1],
                            in1=_f32(seqT[b][:, c, :]), op0=AL.mult, op1=AL.add)
                        nc.vector.tensor_scalar(
                            out=baseT[:, c, :], in0=_f32(baseT[:, c, :]),
                            scalar1=noc[:, c:c + 1], scalar2=None, op0=AL.add)

                # ---- phase B: 5 scans ----
                def fill_ax(axt, src_big, gp=True):
                    for c in range(DC):
                        nc.vector.memset(axt[:, c, 0:SPP:SPP - 1], 0.0)
                        tt(axt[:, c, 1:SP + 1], aB, _f32(src_big[:, c, :]),
                           AL.mult, gp=gp)

                # lcT first: unblocks w1/w2 on PE while the l1/l2 chain runs
                lcT = work_big(f"lcT{b}", tag="lcT", dtype=R)
                axB = work.tile([128, DC, SPP], F32, name=f"axB{b}", tag="axT")
                fill_ax(axB, seqT[b])
                for c in range(DC):
                    scan_fwd(lcT[:, c, :], naB, axB[:, c])

                # deferred D/F/G tail of the previous batch (its tsc/comp are
                # ready by now, so these row chains run stall-free while this
                # batch's matmuls occupy PE)
                if len(pending_tail) >= 1:
                    emit_tail(*pending_tail.pop(0))

                # w1 -> gelu -> interT issued early on PE
                interT = work.tile([128, 8, SP], W2R, name=f"interT{b}",
                                   tag="interT")
                cc_rhs = [lcT[:, 0, :], lcT[:, 1, :],
                          seqT[b][:, 0, :], seqT[b][:, 1, :]]
                for hk in range(8):
                    ps = psum.tile([128, SP], F32, name=f"ps_w1{b}{hk}",
                                   tag="psmm", bufs=2)
                    mm(ps, [w1W_t[k][:, hk * 128:(hk + 1) * 128] for k in range(4)],
                       cc_rhs)
                    gelu_act(interT[:, hk, :], ps,
                             w1b_t[:, hk:hk + 1] if w1b_t is not None else None)

                fill_ax(axB, baseT)
                l1T = work_big(f"l1T{b}", tag="l1T", dtype=R)
                r1T = work_big(f"r1T{b}", tag="r1T", dtype=R)
                for c in range(DC):
                    scan_fwd(l1T[:, c, :], naB, axB[:, c])
                    scan_bwd(r1T[:, c, :], naB, axB[:, c])
                l2T = work_big(f"l2T{b}", tag="l2T", dtype=R)
                r2T = work_big(f"r2T{b}", tag="r2T", dtype=R)
                ax2 = work.tile([128, DC, SPP], F32, name=f"ax2{b}", tag="axT")
                fill_ax(ax2, l1T)
                for c in range(DC):
                    scan_fwd(l2T[:, c, :], naB, ax2[:, c])
                fill_ax(ax2, r1T)
                for c in range(DC):
                    scan_bwd(r2T[:, c, :], naB, ax2[:, c])


                # ---- phase C: conv (transposed) + score ----
                # contract in piece-readiness order so PE starts as soon as
                # baseT/l1T/r1T exist instead of waiting for the l2T chain
                piece_order = [(2, baseT), (1, l1T), (3, r1T), (0, l2T), (4, r2T)]
                gT = work_big(f"gT{b}", tag="gpar", dtype=R, bufs=2)
                for c in range(DC):
                    ps = psum.tile([128, SP], F32, name=f"ps_cv{b}{c}", tag="psmm", bufs=2)
                    lhsT, rhs = [], []
                    for w, piece in piece_order:
                        for ci in range(DC):
                            lhsT.append(convW_t[w * DC + ci][:, c * 128:(c + 1) * 128])
                            rhs.append(piece[:, ci, :])
                    mm(ps, lhsT, rhs)
                    gelu_act(gT[:, c, :], ps,
                             convb_t[:, c:c + 1] if convb_t is not None else None)
                ps_tsc = psum.tile([1, SP], F32, name=f"ps_tsc{b}", tag="psrow", bufs=2)
                mm(ps_tsc, [scW_t[:, c:c + 1] for c in range(DC)],
                   [gT[:, c, :] for c in range(DC)])
                tsc = row(f"tsc{b}")
                if scb_t is not None:
                    nc.scalar.activation(out=tsc, in_=ps_tsc, func=AF.Identity,
                                         bias=scb_t[0:1, 0:1])
                else:
                    nc.scalar.activation(out=tsc, in_=ps_tsc, func=AF.Copy)

                # ---- phase E: w2 -> gated sum ----
                comp = work_big(f"comp{b}", tag="compT", dtype=R, bufs=3)
                parT = work_big(f"parT{b}", tag="gpar", bufs=2)
                inter_lhsT = [interT[:, hk, :] for hk in range(8)]
                for g in [3, 0, 1, 2]:
                    for c in range(DC):
                        cc = g * DC + c
                        ps = psum.tile([128, SP], F32, name=f"ps_w2{b}{cc}",
                                       tag="psmm", bufs=2)
                        mm(ps, [w2W_t[hk][:, cc * 128:(cc + 1) * 128]
                                for hk in range(8)], inter_lhsT)
                        if g == 3:
                            if w2b_t is not None:
                                nc.scalar.activation(out=parT[:, c, :], in_=ps,
                                                     func=AF.Identity,
                                                     bias=w2b_t[:, cc:cc + 1])
                            else:
                                nc.scalar.activation(out=parT[:, c, :], in_=ps,
                                                     func=AF.Copy)
                        else:
                            bias = w2b_t[:, cc:cc + 1] if w2b_t is not None else 0.0
                            gate = work.tile([128, SP], F32, name=f"gate{b}",
                                             tag="gate", bufs=3)
                            nc.scalar.activation(out=gate, in_=ps, func=AF.Sigmoid,
                                                 bias=bias)
                            src = [lcT, seqT[b], parT][g]
                            if g == 0:
                                nc.vector.tensor_tensor(out=comp[:, c, :], in0=gate,
                                                        in1=_f32(src[:, c, :]),
                                                        op=AL.mult)
                            else:
                                gm = work.tile([128, SP], F32, name=f"gm{b}",
                                               tag="gelu_tmp2")
                                tt(gm, gate, _f32(src[:, c, :]), AL.mult, gp=True)
                                nc.vector.tensor_tensor(out=comp[:, c, :],
                                                        in0=_f32(comp[:, c, :]),
                                                        in1=gm, op=AL.add)

                pending_tail.append((b, tsc, comp))


        while pending_tail:
            emit_tail(*pending_tail.pop(0))

        # ---------------- output ------------------------------------------
        for b in range(NB):
            for c in range(DC):
                nc.sync.dma_start(out=out_dram.ap()[b, c],
                                  in_=_f32(seqT[b][:, c, 0:S2]))
    return nc


def _host_prep(inputs):
    f32 = np.float32
    seq = np.asarray(inputs["sequence"], f32)
    im = np.asarray(inputs["input_mask"], f32)
    START = np.asarray(inputs["START"], f32)
    END = np.asarray(inputs["END"], f32)
    yes_t = np.asarray(inputs["yes_t"], f32).reshape(-1)
    no_t = np.asarray(inputs["no_t"], f32).reshape(-1)
    N, S, Dd = seq.shape
    assert (N, S, Dd) == (32, 512, 256), (N, S, Dd)

    ones = np.ones((N, 1, 1), f32)
    zeros = np.zeros((N, 1, 1), f32)
    mask0 = np.concatenate([ones, im], 1)
    mask_no_end = np.concatenate([mask0, zeros], 1)
    mask_yes_end = np.concatenate([ones, mask0], 1)
    END_mask = mask_yes_end - mask_no_end
    seqA = np.concatenate([np.broadcast_to(START, (N, 1, Dd)), seq,
                           np.zeros((N, 1, Dd), f32)], 1)
    seqA = (END_mask * END + (1.0 - END_mask) * seqA).astype(f32)
    mask = mask_yes_end
    mask_no_start = np.concatenate([zeros, mask[:, 1:]], 1)
    last_tok = np.concatenate([END_mask[:, 1:], zeros], 1)
    selp = (mask_no_start * mask_no_end * (1.0 - last_tok)).astype(f32)

    seqT = np.zeros((N, DC, 128, SP), f32)
    for c in range(DC):
        seqT[:, c, :, :S2] = seqA[:, :, c * 128:(c + 1) * 128].transpose(0, 2, 1)
    maskP = np.zeros((N, SP), f32)
    maskP[:, :S2] = mask[:, :, 0]
    selpP = np.zeros((N, SP), f32)
    selpP[:, :S2] = selp[:, :, 0]
    actP = maskP.copy()
    nactP = (1.0 - actP).astype(f32)

    def chunk_col(v, nch):
        return np.ascontiguousarray(np.asarray(v, f32).reshape(nch, 128).T)

    host = {
        "seqT": seqT, "mask": maskP, "selp": selpP, "act0": actP, "nact0": nactP,
        "itW": np.asarray(inputs["itW"], f32),
        "convW": np.asarray(inputs["convW"], f32),
        "scWc": chunk_col(np.asarray(inputs["scW"], f32).reshape(-1), DC),
        "w1W": np.asarray(inputs["w1W"], f32),
        "w2W": np.asarray(inputs["w2W"], f32),
        "noc": chunk_col(no_t, DC),
        "ymnc": chunk_col(yes_t - no_t, DC),
    }
    flags = {
        "itbc": bool(np.any(np.asarray(inputs["itb"]))),
        "convbc": bool(np.any(np.asarray(inputs["convb"]))),
        "w1bc": bool(np.any(np.asarray(inputs["w1b"]))),
        "w2bc": bool(np.any(np.asarray(inputs["w2b"]))),
        "scbc": bool(np.any(np.asarray(inputs["scb"]))),
        "lngc": bool(np.any(np.asarray(inputs["lnb"])))
        or bool(np.any(np.asarray(inputs["lng"]) != 1.0)),
    }
    flags["lnbc"] = flags["lngc"]
    if flags["itbc"]:
        host["itbc"] = chunk_col(inputs["itb"], DC)
    if flags["convbc"]:
        host["convbc"] = chunk_col(inputs["convb"], DC)
    if flags["w1bc"]:
        host["w1bc"] = chunk_col(inputs["w1b"], 8)
    if flags["w2bc"]:
        host["w2bc"] = chunk_col(inputs["w2b"], 8)
    if flags["scbc"]:
        host["scbc"] = np.asarray(inputs["scb"], f32).reshape(1, 1)
    if flags["lngc"]:
        host["lngc"] = chunk_col(inputs["lng"], DC)
        host["lnbc"] = chunk_col(inputs["lnb"], DC)
    return host, flags


_PROG_CACHE = {}


def kernel(**inputs):
    global LAST_EXEC_NS, LAST_RES
    n_steps = int(inputs["n_steps"])
    host, flags = _host_prep(inputs)

    key = (n_steps, tuple(sorted(flags.items())), MM_DT, W2_DT, SIM, GP_LVL)
    if key not in _PROG_CACHE:
        _PROG_CACHE[key] = _build_program(n_steps, flags)
    nc = _PROG_CACHE[key]

    per_batch = {"seqT", "mask", "selp", "act0", "nact0"}
    in_maps = []
    for k in range(NCORES):
        m = {}
        for name, arr in host.items():
            if name in per_batch:
                m[name] = np.ascontiguousarray(arr[k * NB:(k + 1) * NB])
            else:
                m[name] = arr
        in_maps.append(m)

    if SIM:
        from concourse.bass_interp import CoreSim
        results = []
        for k in range(int(os.environ.get("CRVNN_SIM_CORES", "1"))):
            sim = CoreSim(nc)
            for name, v in in_maps[k].items():
                sim.tensor(name)[:] = v
            sim.simulate()
            results.append(np.array(sim.tensor("out")))
    else:
        from concourse.bass_utils import run_bass_kernel_spmd
        if not getattr(nc, "_waitfix_done", False):
            n = _split_multiwaits(nc)
            nc._waitfix_done = True
        res = run_bass_kernel_spmd(nc, in_maps, list(range(NCORES)), trace=TRACE)
        LAST_EXEC_NS = res.exec_time_ns
        LAST_RES = res
        results = [res.results[k]["out"] for k in range(NCORES)]

    full = np.zeros((32, S2, D), np.float32)
    for k, o in enumerate(results):
        for b in range(NB):
            for c in range(DC):
                full[k * NB + b, :, c * 128:(c + 1) * 128] = o[b, c].T
    return full



# revision 20
# speedup vs baseline: 1.3823x; 1.3823x over previous
"""CRvNN forward kernel for 8x Trainium2 NeuronCores (Bass/Tile), v3.

Strategy
--------
Pure data parallelism: batch 32 -> 4 per core; params replicated; no
collectives.  State is TRANSPOSED (D=256 on partitions as 2x128 chunks,
position on the free axis, padded 514 -> 516).  The S^2 neighbor-prob
matrices are first-order linear recurrences evaluated as tensor_tensor_
scans; they are never materialized.

v3 design (vs the 965us v1 baseline):
- w2 (1024x1024, 60% of PE work) runs in fp8e4 + MatmulPerfMode.DoubleRow:
  one instruction contracts a 256-row pair at 0.5 cyc/col (4x f32r).  The
  w2 weights are host-prescaled by 64 (fp8 range) and 1/64 is folded into
  the PSUM-read activation scale; interT is written fp8 by the w1 gelu.
- Everything else lives in FLOAT16: same 10-bit mantissa as f32r (so
  near-f32r accuracy), but 2-byte, so DVE tensor_tensor runs 2x (327ns
  vs 594 for a (128,516) op).  fp16 range is safe: all tensors here are
  bounded by ~1e3.  w1/conv matmuls run plain fp16 (1 cyc/col, same as
  f32r, zero rhs-quantization error).
- All (1,516) row math (transition probs, active update, LN stats) is
  batched across the 4 local batches as (4,516) tiles.  LN mean/var and
  the score matvec accumulate into bank-aligned PSUM sections
  (4,3,512)+(4,12) via one-hot lhsT selectors, so partition b receives
  batch b directly from PE.  tp uses the algebraic identity
  tp = selp * sigmoid(tsc) (exact up to the reference's 1e-9 EPS), which
  cuts the serial tail chain from ~12 to ~3 ops.
- Step-boundary latency: the tail DMAs its row groups to DRAM and
  immediately issues ALL next-step partition-broadcast loads, so the
  DRAM round trip overlaps the remaining tail math and the next step's
  applies start as soon as their rows land.
- Engine balancing: the l1/r1/l2/r2 scans and some fills run on Pool
  (gpsimd); the rest of the elementwise work stays on DVE at fp16 rates.

This walrus build supports only ONE sync wait per instruction; a
post-scheduling pass splits multi-wait instructions into single-wait NOP
chains.
"""
import os
import sys
from contextlib import ExitStack

import numpy as np

sys.path.insert(0, "/opt/trn_rl_repo")

import ml_dtypes
import bass_rust
import concourse.bass as bass
import concourse.mybir as mybir
from concourse.tile import TileContext

F32 = mybir.dt.float32
F16 = mybir.dt.float16
BF16 = mybir.dt.bfloat16
F8 = mybir.dt.float8e4
AL = mybir.AluOpType
AF = mybir.ActivationFunctionType
PM = mybir.MatmulPerfMode

NCORES = 8
NB = 4            # batch per core
D = 256
DC = 2            # D chunks of 128
S2 = 514
SP = 516          # padded sequence length
SPP = SP + 2      # scan-input tiles have leading+trailing zero pad columns
H = 1024
WIN = 5
EPS = 1e-9
WSCALE = 64.0     # fp8 weight prescale (folded back via activation scale)

SIM = os.environ.get("CRVNN_SIM", "0") == "1"
TRACE = os.environ.get("CRVNN_TRACE", "0") == "1"
# compat attrs (test.py uses these in its program-cache key)
MM_DT = os.environ.get("CRVNN_MMDT", "f8")
W2_DT = os.environ.get("CRVNN_W2DT", "f8")
GP_LVL = int(os.environ.get("CRVNN_GP", "1"))
# per-matmul-group dtype: "f8" = fp8e4 + DoubleRow; anything else = fp16
F8_W1 = os.environ.get("CRVNN_F8W1", "f16") == "f8"
F8_W2 = os.environ.get("CRVNN_F8W2", "f8") == "f8"
F8_CV = os.environ.get("CRVNN_F8CV", "f8") == "f8"

NSPLITS = [(0, 512), (512, SP - 512)]

LAST_EXEC_NS = None
LAST_RES = None

# engine assignment knobs: 'v' = DVE, 'g' = Pool/gpsimd
ENG = {
    "fill_seq": "v",
    "fill_base": "g",
    "fill_l2": "g",
    "fill_r2": "g",
    "scan_lc": "v",
    "scan_l1": "v",
    "scan_r1": "v",
    "scan_l2": "v",
    "scan_r2": "v",
    "gm": "g",
    "t2": "g",
    "sq": "g",
}


# --------------------------------------------------------------------------
# post-scheduling fixup: split multi-wait instructions into 1-wait NOP chains
# --------------------------------------------------------------------------
def _split_multiwaits(nc):
    counter = [0]

    def mk_nop(engine, wait):
        counter[0] += 1
        n = bass_rust.InstNoOp(name=f"WFIX-{counter[0]}", ins=[], outs=[])
        n.engine = engine
        n.sync_info = bass_rust.SyncInfo(on_wait=[wait], on_update=[])
        return n

    total = 0
    for f in nc.m.functions:
        for bb in f.blocks:
            out = []
            changed = False
            for inst in list(bb.instructions):
                si = inst.sync_info
                waits = list(si.on_wait) if (si is not None and si.on_wait) else []
                if len(waits) > 1:
                    for w in waits[:-1]:
                        out.append(mk_nop(inst.engine, w))
                    inst.sync_info = bass_rust.SyncInfo(
                        on_wait=[waits[-1]],
                        on_update=list(si.on_update) if si.on_update else [])
                    changed = True
                    total += 1
                out.append(inst)
            if changed:
                bb.instructions = out
    return total


def _bcast_ap(drow):
    """DRAM row AP (1, n) -> partition-broadcast AP (128, n)."""
    return bass.AP(tensor=drow.tensor, offset=drow.offset,
                   ap=[[0, 128]] + drow.ap[1:])


def _build_program(n_steps, flags):
    nc = bass.Bass()

    W1T = F8 if F8_W1 else F16
    W2T = F8 if F8_W2 else F16
    CVT = F8 if F8_CV else F16
    w1_scale = 1.0 / WSCALE if F8_W1 else 1.0
    w2_scale = 1.0 / WSCALE if F8_W2 else 1.0
    cv_scale = 1.0 / WSCALE if F8_CV else 1.0

    seqT_in = nc.declare_dram_parameter("seqT", [NB, DC, 128, SP], F16, isOutput=False)
    mask_in = nc.declare_dram_parameter("mask", [NB, SP], F16, isOutput=False)
    selp_in = nc.declare_dram_parameter("selp", [NB, SP], F16, isOutput=False)
    act0_in = nc.declare_dram_parameter("act0", [NB, SP], F16, isOutput=False)
    act0f_in = nc.declare_dram_parameter("act0f", [NB, SP], F32, isOutput=False)
    nact0_in = nc.declare_dram_parameter("nact0", [NB, SP], F16, isOutput=False)
    itW_in = nc.declare_dram_parameter("itW", [D, D], F16, isOutput=False)
    convW_in = nc.declare_dram_parameter("convW", [WIN * D, D], CVT, isOutput=False)
    w1W_in = nc.declare_dram_parameter("w1W", [2 * D, H], W1T, isOutput=False)
    w2W_in = nc.declare_dram_parameter("w2W", [H, 4 * D], W2T, isOutput=False)
    sc4_in = nc.declare_dram_parameter("sc4", [128, NB, DC, 4], F16, isOutput=False)
    ob4_in = nc.declare_dram_parameter("ob4", [128, NB, 4], F16, isOutput=False)
    bsel_in = nc.declare_dram_parameter("bsel", [4, NB, 128], F16, isOutput=False)
    noc_in = nc.declare_dram_parameter("noc", [128, DC], F32, isOutput=False)
    ymn_in = nc.declare_dram_parameter("ymnc", [128, DC], F32, isOutput=False)
    opt_in = {}
    for nm, shape in [("itbc", [128, DC]), ("convbc", [128, DC]),
                      ("w1bc", [128, 8]), ("w2bc", [128, 8]), ("scbc", [4, 1]),
                      ("lngc", [128, DC]), ("lnbc", [128, DC])]:
        if flags.get(nm):
            opt_in[nm] = nc.declare_dram_parameter(nm, shape, F32, isOutput=False)
    out_dram = nc.declare_dram_parameter("out", [NB, DC, 128, S2], F32, isOutput=True)

    with TileContext(nc) as tc, ExitStack() as ctx:
        wpool = ctx.enter_context(tc.tile_pool(name="wpool", bufs=1))
        state = ctx.enter_context(tc.tile_pool(name="state", bufs=1))
        work = ctx.enter_context(tc.tile_pool(name="work", bufs=1))
        psum = ctx.enter_context(tc.tile_pool(name="psum", bufs=1, space="PSUM"))
        dram = ctx.enter_context(tc.tile_pool(name="dramp", bufs=1, space="DRAM"))

        V = nc.vector
        G = nc.gpsimd

        def eng(key):
            return G if ENG[key] == "g" else V

        # ---------------- weights -> SBUF ----------------------------------
        # pair tiles: (128, 2, M); [:, i, :] = rows [p*256 + i*128 : +128]
        # (host already converted to the matmul dtype)
        def load_pairs(name, dram_p, n_pairs, M, dt):
            tiles = []
            for p in range(n_pairs):
                t = wpool.tile([128, 2, M], dt, name=f"{name}{p}")
                nc.sync.dma_start(
                    out=t,
                    in_=dram_p.ap()[p * 256:(p + 1) * 256, :].rearrange(
                        "(two q) m -> q two m", two=2))
                tiles.append(t)
            return tiles

        w1W8 = load_pairs("w1W8", w1W_in, 2, H, W1T)
        w2W8 = load_pairs("w2W8", w2W_in, 4, 4 * D, W2T)
        cvW8 = load_pairs("cvW8", convW_in, 5, D, CVT)

        itW_t = wpool.tile([128, 2, D], F16, name="itWt")
        nc.sync.dma_start(out=itW_t,
                          in_=itW_in.ap().rearrange("(two q) m -> q two m", two=2))
        sc4 = wpool.tile([128, NB, DC, 4], F16, name="sc4t")
        nc.sync.dma_start(out=sc4, in_=sc4_in.ap())
        ob4 = wpool.tile([128, NB, 4], F16, name="ob4t")
        nc.sync.dma_start(out=ob4, in_=ob4_in.ap())
        bsel = wpool.tile([4, NB, 128], F16, name="bselt")
        nc.sync.dma_start(out=bsel, in_=bsel_in.ap())
        noc = wpool.tile([128, DC], F32, name="noct")
        nc.sync.dma_start(out=noc, in_=noc_in.ap())
        ymnc = wpool.tile([128, DC], F32, name="ymnct")
        nc.sync.dma_start(out=ymnc, in_=ymn_in.ap())
        eps4 = wpool.tile([4, 1], F32, name="eps4")
        nc.vector.memset(eps4, 1e-5)

        def load_opt(nm, shape):
            if nm not in opt_in:
                return None
            t = wpool.tile(shape, F32, name=f"{nm}_t")
            nc.sync.dma_start(out=t, in_=opt_in[nm].ap())
            return t

        itb_t = load_opt("itbc", [128, DC])
        convb_t = load_opt("convbc", [128, DC])
        w1b_t = load_opt("w1bc", [128, 8])
        w2b_t = load_opt("w2bc", [128, 8])
        scb_t = load_opt("scbc", [4, 1])
        lng_t = load_opt("lngc", [128, DC])
        lnb_t = load_opt("lnbc", [128, DC])

        # ---------------- persistent state ---------------------------------
        seqT = [state.tile([128, DC, SP], F16, name=f"seqT{b}") for b in range(NB)]
        if F8_W1:
            seqT8 = [state.tile([128, DC, SP], F8, name=f"seqT8_{b}")
                     for b in range(NB)]
        else:
            seqT8 = seqT
        a4 = state.tile([NB, SP], F32, name="a4")
        nc.sync.dma_start(out=a4, in_=act0f_in.ap())
        mask4 = state.tile([NB, SP], F16, name="mask4")
        nc.sync.dma_start(out=mask4, in_=mask_in.ap())
        selp4 = state.tile([NB, SP], F16, name="selp4")
        nc.sync.dma_start(out=selp4, in_=selp_in.ap())

        # PSUM: matmul tiles + batched-stat sections
        def psmm(name):
            return psum.tile([128, SP], F32, name=name, tag="psmm", bufs=2)

        ps_big = psum.tile([4, 3, 512], F32, name="ps_big", tag="psbig", bufs=1)
        ps_tail = psum.tile([4, 12], F32, name="ps_tail", tag="pstail", bufs=1)

        def row4(name, dt=F32, bufs=6):
            return work.tile([NB, SP], dt, name=name, tag="row4", bufs=bufs)

        def bc_tile(name, tag, bufs):
            return work.tile([128, SP], F16, name=name, tag=tag, bufs=bufs)

        def bounce_bcast(drow_b, name, tag, bufs=5):
            """(1,SP) slice of a DRAM (4,SP) tile -> (128,SP) bcast tile."""
            t = bc_tile(name, tag=tag, bufs=bufs)
            nc.sync.dma_start(out=t, in_=_bcast_ap(drow_b))
            return t

        def napad(name, src_ap):
            """(128, SPP) bcast tile with zero pads at cols 0, SPP-1."""
            t = work.tile([128, SPP], F16, name=name, tag="nabP", bufs=5)
            nc.vector.memset(t[:, 0:SPP:SPP - 1], 0.0)
            nc.sync.dma_start(out=t[:, 1:SP + 1], in_=src_ap)
            return t

        def pe_bcast(row, b, name, tag, copy_eng, pads=False, bufs=5):
            """Broadcast row b of a (4,SP) SBUF tile to (128,SP) via PE:
            psum[p,i] = sum_q bsel[q,b,p]*row[q,i] = row[b,i], then one
            engine copy PSUM->SBUF.  No DRAM round trip."""
            ps = psmm(f"bc_{name}")
            for (o, s) in NSPLITS:
                nc.tensor.matmul(ps[:, o:o + s], bsel[:, b, :],
                                 row[:, o:o + s], start=True, stop=True)
            if pads:
                t = work.tile([128, SPP], F16, name=name, tag="nabP",
                              bufs=bufs)
                nc.vector.memset(t[:, 0:SPP:SPP - 1], 0.0)
                dst = t[:, 1:SP + 1]
            else:
                t = bc_tile(name, tag=tag, bufs=bufs)
                dst = t
            if copy_eng == "act":
                nc.scalar.activation(out=dst, in_=ps, func=AF.Copy)
            elif copy_eng == "pool":
                nc.gpsimd.tensor_scalar(out=dst, in0=ps, scalar1=1.0,
                                        scalar2=None, op0=AL.mult)
            else:
                nc.vector.tensor_scalar(out=dst, in0=ps, scalar1=1.0,
                                        scalar2=None, op0=AL.mult)
            return t

        def ax_tile(name, tag):
            t = work.tile([128, DC, SPP], F16, name=name, tag=tag,
                          bufs=(3 if tag == "axs" else 2))
            for c in range(DC):
                nc.vector.memset(t[:, c, 0:SPP:SPP - 1], 0.0)
            return t

        def scan_fwd(e, out_c, nap, datap):
            """out[i] = data[i-1] + na[i-1]*out[i-1]; data pad supplies z0=0."""
            e.tensor_tensor_scan(
                out=out_c, data0=nap[:, 0:SP], data1=datap[:, 0:SP],
                initial=0.0, op0=AL.mult, op1=AL.add)

        def scan_bwd(e, out_c, nap, datap):
            e.tensor_tensor_scan(
                out=out_c[:, ::-1], data0=nap[:, SPP - 1:1:-1],
                data1=datap[:, SPP - 1:1:-1], initial=0.0,
                op0=AL.mult, op1=AL.add)

        def gelu_act(out, in_, bias, scale=1.0):
            b = bias if bias is not None else 0.0
            if SIM:
                n = out.shape[-1]
                x2 = work.tile([out.shape[0], n], F32, name="gx2",
                               tag="gelu_tmp", bufs=2)
                nc.scalar.activation(out=x2, in_=in_, func=AF.Square, bias=b,
                                     scale=scale)
                nc.vector.tensor_scalar(out=x2, in0=x2, scalar1=0.044715,
                                        scalar2=1.0, op0=AL.mult, op1=AL.add)
                u = work.tile([out.shape[0], n], F32, name="gu",
                              tag="gelu_tmp2", bufs=2)
                nc.scalar.activation(out=u, in_=in_, func=AF.Identity, bias=b,
                                     scale=scale)
                nc.vector.tensor_tensor(out=x2, in0=x2, in1=u, op=AL.mult)
                nc.scalar.activation(out=x2, in_=x2, func=AF.Tanh,
                                     scale=0.7978845608028654)
                nc.vector.tensor_scalar(out=x2, in0=x2, scalar1=1.0,
                                        scalar2=0.5, op0=AL.add, op1=AL.mult)
                nc.vector.tensor_tensor(out=out, in0=x2, in1=u, op=AL.mult)
            else:
                nc.scalar.activation(out=out, in_=in_, func=AF.Gelu_apprx_tanh,
                                     bias=b, scale=scale)

        # matmul helper: lhsT pair tiles, rhs (128, 2, s) slices
        def mmdr(ps_ap, pairs, f8, nsl=NSPLITS):
            K = len(pairs)
            for (o, s) in nsl:
                for k, (lhsT, rhs) in enumerate(pairs):
                    if f8:
                        nc.tensor.matmul(ps_ap[:, o:o + s], lhsT,
                                         rhs[:, :, o:o + s],
                                         start=(k == 0), stop=(k == K - 1),
                                         perf_mode=PM.DoubleRow)
                    else:
                        for i in range(2):
                            nc.tensor.matmul(ps_ap[:, o:o + s], lhsT[:, i, :],
                                             rhs[:, i, o:o + s],
                                             start=(k == 0 and i == 0),
                                             stop=(k == K - 1 and i == 1))

        # batched-stat matmul into ps_big/ps_tail section t.  ps_tail's three
        # sections share one PSUM bank = one accumulation group per step.
        def mm_stat(t, lhsT, rhs, start, stop, tail_start, tail_stop,
                    tail=True):
            nc.tensor.matmul(ps_big[:, t, :], lhsT, rhs[:, 0:512],
                             start=start, stop=stop)
            if tail:
                nc.tensor.matmul(ps_tail[:, 4 * t:4 * t + 4], lhsT,
                                 rhs[:, 512:SP],
                                 start=tail_start, stop=tail_stop)

        def read_stat(t, name, bias=None, scale=1.0, dt=F32, tail=True):
            r = row4(name, dt=dt)
            func = AF.Copy if bias is None else AF.Identity
            b = 0.0 if bias is None else bias
            nc.scalar.activation(out=r[:, 0:512], in_=ps_big[:, t, :],
                                 func=func, bias=b, scale=scale)
            if tail:
                nc.scalar.activation(out=r[:, 512:SP],
                                     in_=ps_tail[:, 4 * t:4 * t + 4],
                                     func=func, bias=b, scale=scale)
            else:
                # tsc cols >= 512 are always selp-masked to zero downstream
                nc.vector.memset(r[:, 512:SP], 0.0)
            return r

        # ------------------------------------------------------------------
        # apply: seq_new = rA*pre - rB [ *lng + gate*lnb ] (+ rC*seq_old)
        # ------------------------------------------------------------------
        def apply_ln(b, pre, bc, dst, mk_shadow, per_chunk_dma=None):
            rAB, rBB, rCB, gateB = bc
            for c in range(DC):
                t1 = work.tile([128, SP], F16, name="t1g", tag="t1g", bufs=2)
                nc.vector.tensor_tensor(out=t1, in0=rAB, in1=pre[:, c, :],
                                        op=AL.mult)
                nc.vector.tensor_tensor(out=t1, in0=t1, in1=rBB, op=AL.subtract)
                if lng_t is not None:
                    nc.vector.tensor_scalar(out=t1, in0=t1,
                                            scalar1=lng_t[:, c:c + 1],
                                            scalar2=None, op0=AL.mult)
                    nc.vector.scalar_tensor_tensor(
                        out=t1, in0=gateB, scalar=lnb_t[:, c:c + 1], in1=t1,
                        op0=AL.mult, op1=AL.add)
                if rCB is None:
                    nc.vector.tensor_copy(out=dst[b][:, c, :], in_=t1)
                else:
                    t2 = work.tile([128, SP], F16, name="t2g", tag="t2g", bufs=2)
                    eng("t2").tensor_tensor(out=t2, in0=rCB,
                                            in1=seqT[b][:, c, :], op=AL.mult)
                    nc.vector.tensor_tensor(out=dst[b][:, c, :], in0=t1,
                                            in1=t2, op=AL.add)
                if mk_shadow and F8_W1:
                    nc.vector.tensor_scalar(out=seqT8[b][:, c, :],
                                            in0=seqT[b][:, c, :],
                                            scalar1=1.0, scalar2=None,
                                            op0=AL.mult)
                if per_chunk_dma is not None:
                    per_chunk_dma(b, c, dst[b])

        # prefetched broadcast tiles for the next step, keyed per batch
        bc_next = {}

        def prefetch_apply_bc(suffix, rA, rB, rC, tpm):
            for b in range(NB):
                rAB = pe_bcast(rA, b, f"rAB{suffix}{b}", "rABt", "act")
                rBB = pe_bcast(rB, b, f"rBB{suffix}{b}", "rBBt", "act")
                rCB = (pe_bcast(rC, b, f"rCB{suffix}{b}", "rCBt", "dve")
                       if rC is not None else None)
                tpmB = None
                if lng_t is not None:
                    tpmB = pe_bcast(tpm, b, f"tpmB{suffix}{b}", "tpmBt", "act")
                bc_next[b] = (rAB, rBB, rCB, tpmB)

        def prefetch_row_bc(suffix, a_f, na_f, ltp):
            for b in range(NB):
                aB = pe_bcast(a_f, b, f"aB{suffix}{b}", "aBt", "act")
                naB = pe_bcast(na_f, b, f"naB{suffix}{b}", None, "pool",
                               pads=True)
                ltpB = (pe_bcast(ltp, b, f"ltpB{suffix}{b}", "ltpBt", "dve")
                        if ltp is not None else None)
                bc_next[b] = bc_next[b] + (aB, naB, ltpB)

        # ================= initial transform ================================
        pre_t = []
        for b in range(NB):
            sA = work.tile([128, DC, SP], F16, name=f"sA{b}", tag="sA", bufs=2)
            nc.sync.dma_start(out=sA,
                              in_=seqT_in.ap()[b].rearrange("c p i -> p c i"))
            pre = work.tile([128, DC, SP], F16, name=f"pre{b}", tag="compT",
                            bufs=NB)
            for c in range(DC):
                ps = psmm(f"ps_pre{b}{c}")
                for (o, s) in NSPLITS:
                    for k in range(2):
                        nc.tensor.matmul(ps[:, o:o + s],
                                         itW_t[:, k, c * 128:(c + 1) * 128],
                                         sA[:, k, o:o + s],
                                         start=(k == 0), stop=(k == 1))
                if itb_t is not None:
                    nc.scalar.activation(out=pre[:, c, :], in_=ps,
                                         func=AF.Identity,
                                         bias=itb_t[:, c:c + 1])
                else:
                    nc.scalar.activation(out=pre[:, c, :], in_=ps, func=AF.Copy)
            for c in range(DC):
                mm_stat(1, ob4[:, b, :], pre[:, c, :],
                        start=(b == 0 and c == 0), stop=(b == NB - 1 and c == 1),
                        tail_start=(b == 0 and c == 0), tail_stop=False)
            for c in range(DC):
                sq = work.tile([128, SP], F16, name=f"sq0_{b}{c}", tag="sq",
                               bufs=2)
                eng("sq").tensor_tensor(out=sq, in0=pre[:, c, :],
                                        in1=pre[:, c, :], op=AL.mult)
                mm_stat(2, ob4[:, b, :], sq,
                        start=(b == 0 and c == 0), stop=(b == NB - 1 and c == 1),
                        tail_start=False,
                        tail_stop=(b == NB - 1 and c == 1))
            pre_t.append(pre)

        def ln_rows():
            """ps sections 1,2 -> (rstd, m) (4,SP) f32 rows; rB = rA*m."""
            m_r = read_stat(1, "m_r", scale=1.0 / D)
            v_r = read_stat(2, "v_r", scale=1.0 / D)
            msq = row4("msq")
            nc.scalar.activation(out=msq, in_=m_r, func=AF.Square)
            nc.vector.tensor_tensor(out=v_r, in0=v_r, in1=msq, op=AL.subtract)
            nc.scalar.activation(out=v_r, in_=v_r, func=AF.Sqrt,
                                 bias=eps4[:, 0:1])
            rstd = row4("rstd")
            nc.vector.reciprocal(out=rstd, in_=v_r)
            return rstd, m_r

        rstd, m_r = ln_rows()
        rA0 = row4("rA0", dt=F16)
        nc.vector.tensor_tensor(out=rA0, in0=rstd, in1=mask4, op=AL.mult)
        rB0 = row4("rB0", dt=F16)
        nc.vector.tensor_tensor(out=rB0, in0=rA0, in1=m_r, op=AL.mult)
        prefetch_apply_bc("i", rA0, rB0, None, mask4)
        for b in range(NB):
            aB = bounce_bcast(act0_in.ap()[b:b + 1, :], f"aBi{b}", "aBt")
            naB = napad(f"naBi{b}", _bcast_ap(nact0_in.ap()[b:b + 1, :]))
            bc_next[b] = bc_next[b] + (aB, naB, None)
        for b in range(NB):
            apply_ln(b, pre_t[b], bc_next[b][:4], seqT, True)
        pre_t = None

        # ================= main steps =======================================
        comp_t = [None] * NB

        lc_t = [None] * NB
        lc8_t = [None] * NB
        base_t = [None] * NB
        scan_t = [None] * NB
        inter_t = [None] * NB

        def phase_a(b, s):
            """apply + baseT + lc scan chain."""
            rAB, rBB, rCB, tpmB, aB, naB, ltpB = bc_next[b]
            if s > 0:
                apply_ln(b, comp_t[b], (rAB, rBB, rCB, tpmB), seqT, True)

            # ---- baseT = seqT + tf ----
            baseT = work.tile([128, DC, SP], CVT, name=f"baseT{b}",
                              tag="baseT", bufs=NB)
            if s == 0:
                for c in range(DC):
                    nc.vector.tensor_scalar(out=baseT[:, c, :],
                                            in0=seqT[b][:, c, :],
                                            scalar1=noc[:, c:c + 1],
                                            scalar2=None, op0=AL.add)
            else:
                for c in range(DC):
                    tfB = work.tile([128, SP], F16, name=f"tfB{b}{c}",
                                    tag="tfB", bufs=3)
                    nc.vector.tensor_scalar(out=tfB, in0=ltpB,
                                            scalar1=ymnc[:, c:c + 1],
                                            scalar2=noc[:, c:c + 1],
                                            op0=AL.mult, op1=AL.add)
                    nc.vector.tensor_tensor(out=baseT[:, c, :], in0=tfB,
                                            in1=seqT[b][:, c, :], op=AL.add)

            base_t[b] = baseT
            # ---- lc scan chain ----
            axB = ax_tile(f"axB{b}", "axB")
            for c in range(DC):
                eng("fill_seq").tensor_tensor(out=axB[:, c, 1:SP + 1], in0=aB,
                                              in1=seqT[b][:, c, :], op=AL.mult)
            lcT = work.tile([128, DC, SP], F16, name=f"lcT{b}", tag="lcT",
                            bufs=NB)
            for c in range(DC):
                scan_fwd(eng("scan_lc"), lcT[:, c, :], naB, axB[:, c])
            if F8_W1:
                lcT8 = work.tile([128, DC, SP], F8, name=f"lcT8_{b}",
                                 tag="lcT8", bufs=NB)
                for c in range(DC):
                    nc.vector.tensor_scalar(out=lcT8[:, c, :],
                                            in0=lcT[:, c, :], scalar1=1.0,
                                            scalar2=None, op0=AL.mult)
            else:
                lcT8 = lcT
            lc_t[b] = lcT
            lc8_t[b] = lcT8

        def phase_b(b, s):
            """w1 matmuls -> interT."""
            lcT8 = lc8_t[b]
            # ---- w1 -> gelu -> interT ----
            interT = work.tile([128, 8, SP], W2T, name=f"interT{b}",
                               tag="interT", bufs=NB)
            for hk in range(8):
                ps = psmm(f"ps_w1{b}{hk}")
                mmdr(ps, [(w1W8[0][:, :, hk * 128:(hk + 1) * 128], lcT8),
                          (w1W8[1][:, :, hk * 128:(hk + 1) * 128], seqT8[b])],
                     F8_W1)
                gelu_act(interT[:, hk, :], ps,
                         w1b_t[:, hk:hk + 1] if w1b_t is not None else None,
                         scale=w1_scale)
            inter_t[b] = interT

        def phase_c(b, s):
            """l1/r1/l2/r2 fills + scans."""
            _, _, _, _, aB, naB, _ = bc_next[b]
            baseT = base_t[b]
            axb = ax_tile(f"axb{b}", "axs")
            for c in range(DC):
                eng("fill_base").tensor_tensor(out=axb[:, c, 1:SP + 1], in0=aB,
                                               in1=baseT[:, c, :], op=AL.mult)
            l1T = work.tile([128, DC, SP], CVT, name=f"l1T{b}", tag="l1T", bufs=NB)
            r1T = work.tile([128, DC, SP], CVT, name=f"r1T{b}", tag="r1T", bufs=NB)
            for c in range(DC):
                scan_fwd(eng("scan_l1"), l1T[:, c, :], naB, axb[:, c])
                scan_bwd(eng("scan_r1"), r1T[:, c, :], naB, axb[:, c])
            ax2 = ax_tile(f"ax2{b}", "axs")
            for c in range(DC):
                eng("fill_l2").tensor_tensor(out=ax2[:, c, 1:SP + 1], in0=aB,
                                             in1=l1T[:, c, :], op=AL.mult)
            l2T = work.tile([128, DC, SP], CVT, name=f"l2T{b}", tag="l2T", bufs=NB)
            for c in range(DC):
                scan_fwd(eng("scan_l2"), l2T[:, c, :], naB, ax2[:, c])
            ax2b = ax_tile(f"ax2b{b}", "axs")
            for c in range(DC):
                eng("fill_r2").tensor_tensor(out=ax2b[:, c, 1:SP + 1], in0=aB,
                                             in1=r1T[:, c, :], op=AL.mult)
            r2T = work.tile([128, DC, SP], CVT, name=f"r2T{b}", tag="r2T", bufs=NB)
            for c in range(DC):
                scan_bwd(eng("scan_r2"), r2T[:, c, :], naB, ax2b[:, c])
            scan_t[b] = (l1T, r1T, l2T, r2T)

        def phase_dc(b, s):
            """conv -> gT -> tsc accumulate."""
            baseT = base_t[b]
            l1T, r1T, l2T, r2T = scan_t[b]
            # ---- conv -> gelu -> gT; tsc accumulate ----
            pieces = [(2, baseT), (1, l1T), (3, r1T), (0, l2T), (4, r2T)]
            for c in range(DC):
                ps = psmm(f"ps_cv{b}{c}")
                mmdr(ps, [(cvW8[w][:, :, c * 128:(c + 1) * 128], piece)
                          for (w, piece) in pieces], F8_CV)
                gT = work.tile([128, SP], F16, name=f"gT{b}{c}", tag="gT",
                               bufs=2)
                gelu_act(gT, ps,
                         convb_t[:, c:c + 1] if convb_t is not None else None,
                         scale=cv_scale)
                mm_stat(0, sc4[:, b, c, :], gT,
                        start=(b == 0 and c == 0), stop=(b == NB - 1 and c == 1),
                        tail_start=False, tail_stop=False, tail=False)

        def phase_dw(b, s):
            """w2 -> gates/parent -> comp -> LN stats."""
            lcT = lc_t[b]
            interT = inter_t[b]
            # ---- w2 -> gates/parent -> comp ----
            comp = work.tile([128, DC, SP], F16, name=f"comp{b}", tag="compT",
                             bufs=NB)
            parT = work.tile([128, DC, SP], F16, name=f"parT{b}", tag="gpar",
                             bufs=2)
            for g in [3, 0, 1, 2]:
                for c in range(DC):
                    cc = g * DC + c
                    ps = psmm(f"ps_w2{b}{cc}")
                    mmdr(ps, [(w2W8[p][:, :, cc * 128:(cc + 1) * 128],
                               interT[:, 2 * p:2 * p + 2, :]) for p in range(4)],
                         F8_W2)
                    bias = w2b_t[:, cc:cc + 1] if w2b_t is not None else 0.0
                    if g == 3:
                        nc.scalar.activation(out=parT[:, c, :], in_=ps,
                                             func=AF.Identity, bias=bias,
                                             scale=w2_scale)
                    else:
                        gate = work.tile([128, SP], F16, name=f"gate{b}",
                                         tag="gate", bufs=2)
                        nc.scalar.activation(out=gate, in_=ps, func=AF.Sigmoid,
                                             bias=bias, scale=w2_scale)
                        src = [lcT, seqT[b], parT][g]
                        if g == 0:
                            nc.vector.tensor_tensor(out=comp[:, c, :], in0=gate,
                                                    in1=src[:, c, :], op=AL.mult)
                        else:
                            gm = work.tile([128, SP], F16, name=f"gm{b}",
                                           tag="gmt", bufs=2)
                            eng("gm").tensor_tensor(out=gm, in0=gate,
                                                    in1=src[:, c, :], op=AL.mult)
                            nc.vector.tensor_tensor(out=comp[:, c, :],
                                                    in0=comp[:, c, :],
                                                    in1=gm, op=AL.add)
            comp_t[b] = comp

            # ---- LN stats of comp ----
            for c in range(DC):
                mm_stat(1, ob4[:, b, :], comp[:, c, :],
                        start=(b == 0 and c == 0), stop=(b == NB - 1 and c == 1),
                        tail_start=(b == 0 and c == 0), tail_stop=False)
            for c in range(DC):
                sq = work.tile([128, SP], F16, name=f"sq{b}{c}", tag="sq",
                               bufs=2)
                eng("sq").tensor_tensor(out=sq, in0=comp[:, c, :],
                                        in1=comp[:, c, :], op=AL.mult)
                mm_stat(2, ob4[:, b, :], sq,
                        start=(b == 0 and c == 0), stop=(b == NB - 1 and c == 1),
                        tail_start=False,
                        tail_stop=(b == NB - 1 and c == 1))

        def tail_tp(s):
            """tp/active rows; needs only the tsc stats (conv phase) -> runs
            concurrently with the w2 phase."""
            last = (s == n_steps - 1)
            if not last:
                asq = row4("asq")
                nc.vector.tensor_tensor(out=asq, in0=a4, in1=a4, op=AL.mult)

            # tp = selp * sigmoid(tsc): the reference's
            # tp = e^{t-mx}selp/(e^{t-mx}selp + e^{-mx} + EPS) equals this up
            # to O(EPS); scores are O(1) so no overflow concern.
            tsc = read_stat(0, "tsc", tail=False,
                            bias=scb_t[:, 0:1] if scb_t is not None else None)
            sig = row4("sig", dt=F16)
            nc.scalar.activation(out=sig, in_=tsc, func=AF.Sigmoid)
            tpp = work.tile([NB, SPP], F16, name="tpp", tag="rowP", bufs=2)
            nc.vector.memset(tpp[:, 0:SPP:SPP - 1], 0.0)
            tp = tpp[:, 1:SP + 1]
            nc.vector.tensor_tensor(out=tp, in0=sig, in1=selp4, op=AL.mult)
            tpm = row4("tpm", dt=F16)
            nc.vector.tensor_tensor(out=tpm, in0=tp, in1=mask4, op=AL.mult)
            rC = row4("rC", dt=F16)
            nc.vector.tensor_tensor(out=rC, in0=mask4, in1=tpm, op=AL.subtract)

            if last:
                return tpm, rC, None, None, None

            # active update: a_new = clip(a - a^2*u, 0, 1)*mask
            nap = work.tile([NB, SPP], F16, name="nap", tag="rowP", bufs=2)
            nc.vector.memset(nap[:, 0:SPP:SPP - 1], 0.0)
            nc.vector.tensor_scalar(out=nap[:, 1:SP + 1], in0=a4,
                                    scalar1=-1.0, scalar2=1.0,
                                    op0=AL.mult, op1=AL.add)
            u = row4("u")
            nc.vector.tensor_tensor_scan(
                out=u[:, ::-1], data0=nap[:, SPP - 1:1:-1],
                data1=tpp[:, SPP - 1:1:-1], initial=0.0,
                op0=AL.mult, op1=AL.add)
            nd = row4("nd")
            nc.vector.tensor_tensor(out=nd, in0=asq, in1=u, op=AL.mult)
            nc.vector.tensor_tensor(out=nd, in0=a4, in1=nd, op=AL.subtract)
            nc.vector.tensor_scalar(out=nd, in0=nd, scalar1=0.0,
                                    scalar2=1.0, op0=AL.max, op1=AL.min)
            nc.vector.tensor_tensor(out=a4, in0=nd, in1=mask4, op=AL.mult)
            a_f = row4("a_f", dt=F16)
            nc.vector.tensor_scalar(out=a_f, in0=a4, scalar1=1.0,
                                    scalar2=None, op0=AL.mult)
            na_f = row4("na_f", dt=F16)
            nc.vector.tensor_scalar(out=na_f, in0=a4, scalar1=-1.0,
                                    scalar2=1.0, op0=AL.mult, op1=AL.add)
            return tpm, rC, a_f, na_f, tp

        def tail_bc(s, rows):
            """row broadcasts at the step boundary: PE is idle there and the
            rows were computed during the w2 phase."""
            tpm, rC, a_f, na_f, tp = rows
            for b in range(NB):
                rCB = pe_bcast(rC, b, f"rCBs{s}{b}", "rCBt", "act")
                tpmB = (pe_bcast(tpm, b, f"tpmBs{s}{b}", "tpmBt", "act")
                        if lng_t is not None else None)
                if a_f is None:
                    bc_next[b] = (rCB, tpmB)
                else:
                    aB = pe_bcast(a_f, b, f"aBs{s}{b}", "aBt", "act")
                    naB = pe_bcast(na_f, b, f"naBs{s}{b}", None, "dve",
                                   pads=True)
                    ltpB = pe_bcast(tp, b, f"ltpBs{s}{b}", "ltpBt", "dve")
                    bc_next[b] = (rCB, tpmB, aB, naB, ltpB)

        def tail_ln(s, tpm):
            """LN gating rows; needs the w2-phase mean/var stats."""
            rstd, m_r = ln_rows()
            rA = row4("rA", dt=F16)
            nc.vector.tensor_tensor(out=rA, in0=tpm, in1=rstd, op=AL.mult)
            rB = row4("rB", dt=F16)
            nc.vector.tensor_tensor(out=rB, in0=rA, in1=m_r, op=AL.mult)
            for b in range(NB):
                rAB = pe_bcast(rA, b, f"rABs{s}{b}", "rABt", "dve")
                rBB = pe_bcast(rB, b, f"rBBs{s}{b}", "rBBt", "act")
                bc_next[b] = (rAB, rBB) + bc_next[b]

        for s in range(n_steps):
            for b in range(NB):
                phase_a(b, s)
            for b in range(NB):
                phase_b(b, s)
            for b in range(NB):
                phase_c(b, s)
            for b in range(NB):
                phase_dc(b, s)
            rows = tail_tp(s)
            for b in range(NB):
                phase_dw(b, s)
            tail_bc(s, rows)
            tail_ln(s, rows[0])

        # ---------------- final apply (f32 out) + DMA ----------------------
        outF = [work.tile([128, DC, SP], F32, name=f"outF{b}", tag="outF",
                          bufs=2) for b in range(NB)]

        def out_dma(b, c, dst):
            nc.sync.dma_start(out=out_dram.ap()[b, c], in_=dst[:, c, 0:S2])

        for b in range(NB):
            apply_ln(b, comp_t[b], bc_next[b][:4], outF, False,
                     per_chunk_dma=out_dma)
    return nc


def _host_prep(inputs):
    f32 = np.float32
    f16 = np.float16
    f8 = ml_dtypes.float8_e4m3
    seq = np.asarray(inputs["sequence"], f32)
    im = np.asarray(inputs["input_mask"], f32)
    START = np.asarray(inputs["START"], f32)
    END = np.asarray(inputs["END"], f32)
    yes_t = np.asarray(inputs["yes_t"], f32).reshape(-1)
    no_t = np.asarray(inputs["no_t"], f32).reshape(-1)
    N, S, Dd = seq.shape
    assert (N, S, Dd) == (32, 512, 256), (N, S, Dd)

    ones = np.ones((N, 1, 1), f32)
    zeros = np.zeros((N, 1, 1), f32)
    mask0 = np.concatenate([ones, im], 1)
    mask_no_end = np.concatenate([mask0, zeros], 1)
    mask_yes_end = np.concatenate([ones, mask0], 1)
    END_mask = mask_yes_end - mask_no_end
    seqA = np.concatenate([np.broadcast_to(START, (N, 1, Dd)), seq,
                           np.zeros((N, 1, Dd), f32)], 1)
    seqA = (END_mask * END + (1.0 - END_mask) * seqA).astype(f32)
    mask = mask_yes_end
    mask_no_start = np.concatenate([zeros, mask[:, 1:]], 1)
    last_tok = np.concatenate([END_mask[:, 1:], zeros], 1)
    selp = (mask_no_start * mask_no_end * (1.0 - last_tok)).astype(f32)

    seqT = np.zeros((N, DC, 128, SP), f32)
    for c in range(DC):
        seqT[:, c, :, :S2] = seqA[:, :, c * 128:(c + 1) * 128].transpose(0, 2, 1)
    maskP = np.zeros((N, SP), f32)
    maskP[:, :S2] = mask[:, :, 0]
    selpP = np.zeros((N, SP), f32)
    selpP[:, :S2] = selp[:, :, 0]
    actP = maskP.copy()
    nactP = (1.0 - actP).astype(f32)

    def chunk_col(v, nch):
        return np.ascontiguousarray(np.asarray(v, f32).reshape(nch, 128).T)

    scW = np.asarray(inputs["scW"], f32).reshape(-1)
    sc4 = np.zeros((128, NB, DC, 4), f32)
    for b in range(NB):
        for c in range(DC):
            sc4[:, b, c, b] = scW[c * 128:(c + 1) * 128]
    ob4 = np.zeros((128, NB, 4), f32)
    for b in range(NB):
        ob4[:, b, b] = 1.0
    bsel = np.zeros((4, NB, 128), f32)
    for b in range(NB):
        bsel[b, b, :] = 1.0

    def wconv(name, use_f8):
        w = np.asarray(inputs[name], f32)
        return (w * WSCALE).astype(f8) if use_f8 else w.astype(f16)

    host = {
        "seqT": seqT.astype(f16),
        "mask": maskP.astype(f16), "selp": selpP.astype(f16),
        "act0": actP.astype(f16), "act0f": actP,
        "nact0": nactP.astype(f16),
        "itW": np.asarray(inputs["itW"], f32).astype(f16),
        "convW": wconv("convW", F8_CV),
        "w1W": wconv("w1W", F8_W1),
        "w2W": wconv("w2W", F8_W2),
        "sc4": sc4.astype(f16),
        "ob4": ob4.astype(f16),
        "bsel": bsel.astype(f16),
        "noc": chunk_col(no_t, DC),
        "ymnc": chunk_col(yes_t - no_t, DC),
    }
    flags = {
        "itbc": bool(np.any(np.asarray(inputs["itb"]))),
        "convbc": bool(np.any(np.asarray(inputs["convb"]))),
        "w1bc": bool(np.any(np.asarray(inputs["w1b"]))),
        "w2bc": bool(np.any(np.asarray(inputs["w2b"]))),
        "scbc": bool(np.any(np.asarray(inputs["scb"]))),
        "lngc": bool(np.any(np.asarray(inputs["lnb"])))
        or bool(np.any(np.asarray(inputs["lng"]) != 1.0)),
    }
    flags["lnbc"] = flags["lngc"]
    if flags["itbc"]:
        host["itbc"] = chunk_col(inputs["itb"], DC)
    if flags["convbc"]:
        host["convbc"] = chunk_col(inputs["convb"], DC)
    if flags["w1bc"]:
        host["w1bc"] = chunk_col(inputs["w1b"], 8)
    if flags["w2bc"]:
        host["w2bc"] = chunk_col(inputs["w2b"], 8)
    if flags["scbc"]:
        host["scbc"] = np.broadcast_to(
            np.asarray(inputs["scb"], f32).reshape(1, 1), (4, 1)).copy()
    if flags["lngc"]:
        host["lngc"] = chunk_col(inputs["lng"], DC)
        host["lnbc"] = chunk_col(inputs["lnb"], DC)
    return host, flags


_PROG_CACHE = {}


def kernel(**inputs):
    global LAST_EXEC_NS, LAST_RES
    n_steps = int(inputs["n_steps"])
    host, flags = _host_prep(inputs)

    key = (n_steps, tuple(sorted(flags.items())), MM_DT, W2_DT, SIM, GP_LVL,
           F8_W1, F8_W2, F8_CV)
    if key not in _PROG_CACHE:
        _PROG_CACHE[key] = _build_program(n_steps, flags)
    nc = _PROG_CACHE[key]

    per_batch = {"seqT", "mask", "selp", "act0", "act0f", "nact0"}
    in_maps = []
    for k in range(NCORES):
        m = {}
        for name, arr in host.items():
            if name in per_batch:
                m[name] = np.ascontiguousarray(arr[k * NB:(k + 1) * NB])
            else:
                m[name] = arr
        in_maps.append(m)

    if SIM:
        from concourse.bass_interp import CoreSim
        results = []
        for k in range(int(os.environ.get("CRVNN_SIM_CORES", "1"))):
            sim = CoreSim(nc)
            for name, v in in_maps[k].items():
                sim.tensor(name)[:] = v
            sim.simulate()
            results.append(np.array(sim.tensor("out")))
    else:
        from concourse.bass_utils import run_bass_kernel_spmd
        if not getattr(nc, "_waitfix_done", False):
            _split_multiwaits(nc)
            nc._waitfix_done = True
        res = run_bass_kernel_spmd(nc, in_maps, list(range(NCORES)), trace=TRACE)
        LAST_EXEC_NS = res.exec_time_ns
        LAST_RES = res
        results = [res.results[k]["out"] for k in range(NCORES)]

    full = np.zeros((32, S2, D), np.float32)
    for k, o in enumerate(results):
        for b in range(NB):
            for c in range(DC):
                full[k * NB + b, :, c * 128:(c + 1) * 128] = \
                    np.asarray(o[b, c], np.float32).T
    return full


# revision 22
# speedup vs baseline: 1.4265x; 1.0320x over previous
"""CRvNN forward kernel for 8x Trainium2 NeuronCores (Bass/Tile), v3.

Strategy
--------
Pure data parallelism: batch 32 -> 4 per core; params replicated; no
collectives.  State is TRANSPOSED (D=256 on partitions as 2x128 chunks,
position on the free axis, padded 514 -> 516).  The S^2 neighbor-prob
matrices are first-order linear recurrences evaluated as tensor_tensor_
scans; they are never materialized.

v3 design (vs the 965us v1 baseline):
- w2 (1024x1024, 60% of PE work) runs in fp8e4 + MatmulPerfMode.DoubleRow:
  one instruction contracts a 256-row pair at 0.5 cyc/col (4x f32r).  The
  w2 weights are host-prescaled by 64 (fp8 range) and 1/64 is folded into
  the PSUM-read activation scale; interT is written fp8 by the w1 gelu.
- Everything else lives in FLOAT16: same 10-bit mantissa as f32r (so
  near-f32r accuracy), but 2-byte, so DVE tensor_tensor runs 2x (327ns
  vs 594 for a (128,516) op).  fp16 range is safe: all tensors here are
  bounded by ~1e3.  w1/conv matmuls run plain fp16 (1 cyc/col, same as
  f32r, zero rhs-quantization error).
- All (1,516) row math (transition probs, active update, LN stats) is
  batched across the 4 local batches as (4,516) tiles.  LN mean/var and
  the score matvec accumulate into bank-aligned PSUM sections
  (4,3,512)+(4,12) via one-hot lhsT selectors, so partition b receives
  batch b directly from PE.  tp uses the algebraic identity
  tp = selp * sigmoid(tsc) (exact up to the reference's 1e-9 EPS), which
  cuts the serial tail chain from ~12 to ~3 ops.
- Step-boundary latency: the tail DMAs its row groups to DRAM and
  immediately issues ALL next-step partition-broadcast loads, so the
  DRAM round trip overlaps the remaining tail math and the next step's
  applies start as soon as their rows land.
- Engine balancing: the l1/r1/l2/r2 scans and some fills run on Pool
  (gpsimd); the rest of the elementwise work stays on DVE at fp16 rates.

This walrus build supports only ONE sync wait per instruction; a
post-scheduling pass splits multi-wait instructions into single-wait NOP
chains.
"""
import os
import sys
from contextlib import ExitStack

import numpy as np

sys.path.insert(0, "/opt/trn_rl_repo")

import ml_dtypes
import bass_rust
import concourse.bass as bass
import concourse.mybir as mybir
from concourse.tile import TileContext

F32 = mybir.dt.float32
F16 = mybir.dt.float16
BF16 = mybir.dt.bfloat16
F8 = mybir.dt.float8e4
AL = mybir.AluOpType
AF = mybir.ActivationFunctionType
PM = mybir.MatmulPerfMode

NCORES = 8
NB = 4            # batch per core
D = 256
DC = 2            # D chunks of 128
S2 = 514
SP = 516          # padded sequence length
SPP = SP + 2      # scan-input tiles have leading+trailing zero pad columns
H = 1024
WIN = 5
EPS = 1e-9
WSCALE = 64.0     # fp8 weight prescale (folded back via activation scale)

SIM = os.environ.get("CRVNN_SIM", "0") == "1"
TRACE = os.environ.get("CRVNN_TRACE", "0") == "1"
# compat attrs (test.py uses these in its program-cache key)
MM_DT = os.environ.get("CRVNN_MMDT", "f8")
W2_DT = os.environ.get("CRVNN_W2DT", "f8")
GP_LVL = int(os.environ.get("CRVNN_GP", "1"))
# per-matmul-group dtype: "f8" = fp8e4 + DoubleRow; anything else = fp16
F8_W1 = os.environ.get("CRVNN_F8W1", "f16") == "f8"
F8_W2 = os.environ.get("CRVNN_F8W2", "f8") == "f8"
F8_CV = os.environ.get("CRVNN_F8CV", "f8") == "f8"

NSPLITS = [(0, 512), (512, SP - 512)]

LAST_EXEC_NS = None
LAST_RES = None

# engine assignment knobs: 'v' = DVE, 'g' = Pool/gpsimd
ENG = {
    "fill_seq": "v",
    "fill_base": "g",
    "fill_l2": "g",
    "fill_r2": "g",
    "scan_lc": "v",
    "scan_l1": "v",
    "scan_r1": "v",
    "scan_l2": "v",
    "scan_r2": "v",
    "gm": "g",
    "t2": "g",
    "sq": "g",
}


# --------------------------------------------------------------------------
# post-scheduling fixup: split multi-wait instructions into 1-wait NOP chains
# --------------------------------------------------------------------------
def _split_multiwaits(nc):
    counter = [0]

    def mk_nop(engine, wait):
        counter[0] += 1
        n = bass_rust.InstNoOp(name=f"WFIX-{counter[0]}", ins=[], outs=[])
        n.engine = engine
        n.sync_info = bass_rust.SyncInfo(on_wait=[wait], on_update=[])
        return n

    total = 0
    for f in nc.m.functions:
        for bb in f.blocks:
            out = []
            changed = False
            for inst in list(bb.instructions):
                si = inst.sync_info
                waits = list(si.on_wait) if (si is not None and si.on_wait) else []
                if len(waits) > 1:
                    for w in waits[:-1]:
                        out.append(mk_nop(inst.engine, w))
                    inst.sync_info = bass_rust.SyncInfo(
                        on_wait=[waits[-1]],
                        on_update=list(si.on_update) if si.on_update else [])
                    changed = True
                    total += 1
                out.append(inst)
            if changed:
                bb.instructions = out
    return total


def _bcast_ap(drow):
    """DRAM row AP (1, n) -> partition-broadcast AP (128, n)."""
    return bass.AP(tensor=drow.tensor, offset=drow.offset,
                   ap=[[0, 128]] + drow.ap[1:])


def _build_program(n_steps, flags):
    nc = bass.Bass()

    W1T = F8 if F8_W1 else F16
    W2T = F8 if F8_W2 else F16
    CVT = F8 if F8_CV else F16
    w1_scale = 1.0 / WSCALE if F8_W1 else 1.0
    w2_scale = 1.0 / WSCALE if F8_W2 else 1.0
    cv_scale = 1.0 / WSCALE if F8_CV else 1.0

    seqT_in = nc.declare_dram_parameter("seqT", [NB, DC, 128, SP], F16, isOutput=False)
    mask_in = nc.declare_dram_parameter("mask", [NB, SP], F16, isOutput=False)
    selp_in = nc.declare_dram_parameter("selp", [NB, SP], F16, isOutput=False)
    act0_in = nc.declare_dram_parameter("act0", [NB, SP], F16, isOutput=False)
    act0f_in = nc.declare_dram_parameter("act0f", [NB, SP], F32, isOutput=False)
    nact0_in = nc.declare_dram_parameter("nact0", [NB, SP], F16, isOutput=False)
    itW_in = nc.declare_dram_parameter("itW", [D, D], F16, isOutput=False)
    convW_in = nc.declare_dram_parameter("convW", [WIN * D, D], CVT, isOutput=False)
    w1W_in = nc.declare_dram_parameter("w1W", [2 * D, H], W1T, isOutput=False)
    w2W_in = nc.declare_dram_parameter("w2W", [H, 4 * D], W2T, isOutput=False)
    sc4_in = nc.declare_dram_parameter("sc4", [128, NB, DC, 4], F16, isOutput=False)
    ob4_in = nc.declare_dram_parameter("ob4", [128, NB, 4], F16, isOutput=False)
    bsel_in = nc.declare_dram_parameter("bsel", [4, NB, 128], F16, isOutput=False)
    noc_in = nc.declare_dram_parameter("noc", [128, DC], F32, isOutput=False)
    ymn_in = nc.declare_dram_parameter("ymnc", [128, DC], F32, isOutput=False)
    opt_in = {}
    for nm, shape in [("itbc", [128, DC]), ("convbc", [128, DC]),
                      ("w1bc", [128, 8]), ("w2bc", [128, 8]), ("scbc", [4, 1]),
                      ("lngc", [128, DC]), ("lnbc", [128, DC])]:
        if flags.get(nm):
            opt_in[nm] = nc.declare_dram_parameter(nm, shape, F32, isOutput=False)
    out_dram = nc.declare_dram_parameter("out", [NB, DC, 128, S2], F32, isOutput=True)

    with TileContext(nc) as tc, ExitStack() as ctx:
        wpool = ctx.enter_context(tc.tile_pool(name="wpool", bufs=1))
        state = ctx.enter_context(tc.tile_pool(name="state", bufs=1))
        work = ctx.enter_context(tc.tile_pool(name="work", bufs=1))
        psum = ctx.enter_context(tc.tile_pool(name="psum", bufs=1, space="PSUM"))
        dram = ctx.enter_context(tc.tile_pool(name="dramp", bufs=1, space="DRAM"))

        V = nc.vector
        G = nc.gpsimd

        def eng(key):
            return G if ENG[key] == "g" else V

        # ---------------- weights -> SBUF ----------------------------------
        # pair tiles: (128, 2, M); [:, i, :] = rows [p*256 + i*128 : +128]
        # (host already converted to the matmul dtype)
        def load_pairs(name, dram_p, n_pairs, M, dt):
            tiles = []
            for p in range(n_pairs):
                t = wpool.tile([128, 2, M], dt, name=f"{name}{p}")
                nc.sync.dma_start(
                    out=t,
                    in_=dram_p.ap()[p * 256:(p + 1) * 256, :].rearrange(
                        "(two q) m -> q two m", two=2))
                tiles.append(t)
            return tiles

        w1W8 = load_pairs("w1W8", w1W_in, 2, H, W1T)
        w2W8 = load_pairs("w2W8", w2W_in, 4, 4 * D, W2T)
        cvW8 = load_pairs("cvW8", convW_in, 5, D, CVT)

        itW_t = wpool.tile([128, 2, D], F16, name="itWt")
        nc.sync.dma_start(out=itW_t,
                          in_=itW_in.ap().rearrange("(two q) m -> q two m", two=2))
        sc4 = wpool.tile([128, NB, DC, 4], F16, name="sc4t")
        nc.sync.dma_start(out=sc4, in_=sc4_in.ap())
        ob4 = wpool.tile([128, NB, 4], F16, name="ob4t")
        nc.sync.dma_start(out=ob4, in_=ob4_in.ap())
        bsel = wpool.tile([4, NB, 128], F16, name="bselt")
        nc.sync.dma_start(out=bsel, in_=bsel_in.ap())
        noc = wpool.tile([128, DC], F32, name="noct")
        nc.sync.dma_start(out=noc, in_=noc_in.ap())
        ymnc = wpool.tile([128, DC], F32, name="ymnct")
        nc.sync.dma_start(out=ymnc, in_=ymn_in.ap())
        eps4 = wpool.tile([4, 1], F32, name="eps4")
        nc.vector.memset(eps4, 1e-5)

        def load_opt(nm, shape):
            if nm not in opt_in:
                return None
            t = wpool.tile(shape, F32, name=f"{nm}_t")
            nc.sync.dma_start(out=t, in_=opt_in[nm].ap())
            return t

        itb_t = load_opt("itbc", [128, DC])
        convb_t = load_opt("convbc", [128, DC])
        w1b_t = load_opt("w1bc", [128, 8])
        w2b_t = load_opt("w2bc", [128, 8])
        scb_t = load_opt("scbc", [4, 1])
        lng_t = load_opt("lngc", [128, DC])
        lnb_t = load_opt("lnbc", [128, DC])

        # ---------------- persistent state ---------------------------------
        seqT = [state.tile([128, DC, SP], F16, name=f"seqT{b}") for b in range(NB)]
        if F8_W1:
            seqT8 = [state.tile([128, DC, SP], F8, name=f"seqT8_{b}")
                     for b in range(NB)]
        else:
            seqT8 = seqT
        a4 = state.tile([NB, SP], F32, name="a4")
        nc.sync.dma_start(out=a4, in_=act0f_in.ap())
        mask4 = state.tile([NB, SP], F16, name="mask4")
        nc.sync.dma_start(out=mask4, in_=mask_in.ap())
        selp4 = state.tile([NB, SP], F16, name="selp4")
        nc.sync.dma_start(out=selp4, in_=selp_in.ap())

        # PSUM: matmul tiles + batched-stat sections
        def psmm(name):
            return psum.tile([128, SP], F32, name=name, tag="psmm", bufs=2)

        ps_big = psum.tile([4, 3, 512], F32, name="ps_big", tag="psbig", bufs=1)
        ps_tail = psum.tile([4, 12], F32, name="ps_tail", tag="pstail", bufs=1)

        def row4(name, dt=F32, bufs=8):
            return work.tile([NB, SP], dt, name=name, tag="row4", bufs=bufs)

        def bc_tile(name, tag, bufs):
            return work.tile([128, SP], F16, name=name, tag=tag, bufs=bufs)

        def bounce_bcast(drow_b, name, tag, bufs=5):
            """(1,SP) slice of a DRAM (4,SP) tile -> (128,SP) bcast tile."""
            t = bc_tile(name, tag=tag, bufs=bufs)
            nc.sync.dma_start(out=t, in_=_bcast_ap(drow_b))
            return t

        def napad(name, src_ap):
            """(128, SPP) bcast tile with zero pads at cols 0, SPP-1."""
            t = work.tile([128, SPP], F16, name=name, tag="nabP", bufs=5)
            nc.vector.memset(t[:, 0:SPP:SPP - 1], 0.0)
            nc.sync.dma_start(out=t[:, 1:SP + 1], in_=src_ap)
            return t

        def pe_bcast(row, b, name, tag, copy_eng, pads=False, bufs=5):
            """Broadcast row b of a (4,SP) SBUF tile to (128,SP) via PE:
            psum[p,i] = sum_q bsel[q,b,p]*row[q,i] = row[b,i], then one
            engine copy PSUM->SBUF.  No DRAM round trip."""
            ps = psmm(f"bc_{name}")
            for (o, s) in NSPLITS:
                nc.tensor.matmul(ps[:, o:o + s], bsel[:, b, :],
                                 row[:, o:o + s], start=True, stop=True)
            if pads:
                t = work.tile([128, SPP], F16, name=name, tag="nabP",
                              bufs=bufs)
                nc.vector.memset(t[:, 0:SPP:SPP - 1], 0.0)
                dst = t[:, 1:SP + 1]
            else:
                t = bc_tile(name, tag=tag, bufs=bufs)
                dst = t
            if copy_eng == "act":
                nc.scalar.activation(out=dst, in_=ps, func=AF.Copy)
            elif copy_eng == "pool":
                nc.gpsimd.tensor_scalar(out=dst, in0=ps, scalar1=1.0,
                                        scalar2=None, op0=AL.mult)
            else:
                nc.vector.tensor_scalar(out=dst, in0=ps, scalar1=1.0,
                                        scalar2=None, op0=AL.mult)
            return t

        def ax_tile(name, tag):
            t = work.tile([128, DC, SPP], F16, name=name, tag=tag,
                          bufs=(4 if tag == "axs" else 3))
            for c in range(DC):
                nc.vector.memset(t[:, c, 0:SPP:SPP - 1], 0.0)
            return t

        def scan_fwd(e, out_c, nap, datap):
            """out[i] = data[i-1] + na[i-1]*out[i-1]; data pad supplies z0=0."""
            e.tensor_tensor_scan(
                out=out_c, data0=nap[:, 0:SP], data1=datap[:, 0:SP],
                initial=0.0, op0=AL.mult, op1=AL.add)

        def scan_bwd(e, out_c, nap, datap):
            e.tensor_tensor_scan(
                out=out_c[:, ::-1], data0=nap[:, SPP - 1:1:-1],
                data1=datap[:, SPP - 1:1:-1], initial=0.0,
                op0=AL.mult, op1=AL.add)

        def gelu_act(out, in_, bias, scale=1.0):
            b = bias if bias is not None else 0.0
            if SIM:
                n = out.shape[-1]
                x2 = work.tile([out.shape[0], n], F32, name="gx2",
                               tag="gelu_tmp", bufs=2)
                nc.scalar.activation(out=x2, in_=in_, func=AF.Square, bias=b,
                                     scale=scale)
                nc.vector.tensor_scalar(out=x2, in0=x2, scalar1=0.044715,
                                        scalar2=1.0, op0=AL.mult, op1=AL.add)
                u = work.tile([out.shape[0], n], F32, name="gu",
                              tag="gelu_tmp2", bufs=2)
                nc.scalar.activation(out=u, in_=in_, func=AF.Identity, bias=b,
                                     scale=scale)
                nc.vector.tensor_tensor(out=x2, in0=x2, in1=u, op=AL.mult)
                nc.scalar.activation(out=x2, in_=x2, func=AF.Tanh,
                                     scale=0.7978845608028654)
                nc.vector.tensor_scalar(out=x2, in0=x2, scalar1=1.0,
                                        scalar2=0.5, op0=AL.add, op1=AL.mult)
                nc.vector.tensor_tensor(out=out, in0=x2, in1=u, op=AL.mult)
            else:
                nc.scalar.activation(out=out, in_=in_, func=AF.Gelu_apprx_tanh,
                                     bias=b, scale=scale)

        # matmul helper: lhsT pair tiles, rhs (128, 2, s) slices
        def mmdr(ps_ap, pairs, f8, nsl=NSPLITS):
            K = len(pairs)
            for (o, s) in nsl:
                for k, (lhsT, rhs) in enumerate(pairs):
                    if f8:
                        nc.tensor.matmul(ps_ap[:, o:o + s], lhsT,
                                         rhs[:, :, o:o + s],
                                         start=(k == 0), stop=(k == K - 1),
                                         perf_mode=PM.DoubleRow)
                    else:
                        for i in range(2):
                            nc.tensor.matmul(ps_ap[:, o:o + s], lhsT[:, i, :],
                                             rhs[:, i, o:o + s],
                                             start=(k == 0 and i == 0),
                                             stop=(k == K - 1 and i == 1))

        # batched-stat matmul into ps_big/ps_tail section t.  ps_tail's three
        # sections share one PSUM bank = one accumulation group per step.
        def mm_stat(t, lhsT, rhs, start, stop, tail_start, tail_stop,
                    tail=True):
            nc.tensor.matmul(ps_big[:, t, :], lhsT, rhs[:, 0:512],
                             start=start, stop=stop)
            if tail:
                nc.tensor.matmul(ps_tail[:, 4 * t:4 * t + 4], lhsT,
                                 rhs[:, 512:SP],
                                 start=tail_start, stop=tail_stop)

        def read_stat(t, name, bias=None, scale=1.0, dt=F32, tail=True):
            r = row4(name, dt=dt)
            func = AF.Copy if bias is None else AF.Identity
            b = 0.0 if bias is None else bias
            nc.scalar.activation(out=r[:, 0:512], in_=ps_big[:, t, :],
                                 func=func, bias=b, scale=scale)
            if tail:
                nc.scalar.activation(out=r[:, 512:SP],
                                     in_=ps_tail[:, 4 * t:4 * t + 4],
                                     func=func, bias=b, scale=scale)
            else:
                # tsc cols >= 512 are always selp-masked to zero downstream
                nc.vector.memset(r[:, 512:SP], 0.0)
            return r

        # ------------------------------------------------------------------
        # apply: seq_new = rA*pre - rB [ *lng + gate*lnb ] (+ rC*seq_old)
        # ------------------------------------------------------------------
        def apply_ln(b, pre, bc, dst, mk_shadow, per_chunk_dma=None):
            rAB, rBB, rCB, gateB = bc
            for c in range(DC):
                t1 = work.tile([128, SP], F16, name="t1g", tag="t1g", bufs=3)
                nc.vector.tensor_tensor(out=t1, in0=rAB, in1=pre[:, c, :],
                                        op=AL.mult)
                nc.vector.tensor_tensor(out=t1, in0=t1, in1=rBB, op=AL.subtract)
                if lng_t is not None:
                    nc.vector.tensor_scalar(out=t1, in0=t1,
                                            scalar1=lng_t[:, c:c + 1],
                                            scalar2=None, op0=AL.mult)
                    nc.vector.scalar_tensor_tensor(
                        out=t1, in0=gateB, scalar=lnb_t[:, c:c + 1], in1=t1,
                        op0=AL.mult, op1=AL.add)
                if rCB is None:
                    nc.vector.tensor_copy(out=dst[b][:, c, :], in_=t1)
                else:
                    t2 = work.tile([128, SP], F16, name="t2g", tag="t2g", bufs=3)
                    eng("t2").tensor_tensor(out=t2, in0=rCB,
                                            in1=seqT[b][:, c, :], op=AL.mult)
                    nc.vector.tensor_tensor(out=dst[b][:, c, :], in0=t1,
                                            in1=t2, op=AL.add)
                if mk_shadow and F8_W1:
                    nc.vector.tensor_scalar(out=seqT8[b][:, c, :],
                                            in0=seqT[b][:, c, :],
                                            scalar1=1.0, scalar2=None,
                                            op0=AL.mult)
                if per_chunk_dma is not None:
                    per_chunk_dma(b, c, dst[b])

        # prefetched broadcast tiles for the next step, keyed per batch
        bc_next = {}

        def prefetch_apply_bc(suffix, rA, rB, rC, tpm):
            for b in range(NB):
                rAB = pe_bcast(rA, b, f"rAB{suffix}{b}", "rABt", "act")
                rBB = pe_bcast(rB, b, f"rBB{suffix}{b}", "rBBt", "act")
                rCB = (pe_bcast(rC, b, f"rCB{suffix}{b}", "rCBt", "dve")
                       if rC is not None else None)
                tpmB = None
                if lng_t is not None:
                    tpmB = pe_bcast(tpm, b, f"tpmB{suffix}{b}", "tpmBt", "act")
                bc_next[b] = (rAB, rBB, rCB, tpmB)

        def prefetch_row_bc(suffix, a_f, na_f, ltp):
            for b in range(NB):
                aB = pe_bcast(a_f, b, f"aB{suffix}{b}", "aBt", "act")
                naB = pe_bcast(na_f, b, f"naB{suffix}{b}", None, "pool",
                               pads=True)
                ltpB = (pe_bcast(ltp, b, f"ltpB{suffix}{b}", "ltpBt", "dve")
                        if ltp is not None else None)
                bc_next[b] = bc_next[b] + (aB, naB, ltpB)

        # ================= initial transform ================================
        pre_t = []
        for b in range(NB):
            sA = work.tile([128, DC, SP], F16, name=f"sA{b}", tag="sA", bufs=2)
            nc.sync.dma_start(out=sA,
                              in_=seqT_in.ap()[b].rearrange("c p i -> p c i"))
            pre = work.tile([128, DC, SP], F16, name=f"pre{b}", tag="compT",
                            bufs=NB)
            for c in range(DC):
                ps = psmm(f"ps_pre{b}{c}")
                for (o, s) in NSPLITS:
                    for k in range(2):
                        nc.tensor.matmul(ps[:, o:o + s],
                                         itW_t[:, k, c * 128:(c + 1) * 128],
                                         sA[:, k, o:o + s],
                                         start=(k == 0), stop=(k == 1))
                if itb_t is not None:
                    nc.scalar.activation(out=pre[:, c, :], in_=ps,
                                         func=AF.Identity,
                                         bias=itb_t[:, c:c + 1])
                else:
                    nc.scalar.activation(out=pre[:, c, :], in_=ps, func=AF.Copy)
            for c in range(DC):
                mm_stat(1, ob4[:, b, :], pre[:, c, :],
                        start=(b == 0 and c == 0), stop=(b == NB - 1 and c == 1),
                        tail_start=(b == 0 and c == 0), tail_stop=False)
            for c in range(DC):
                sq = work.tile([128, SP], F16, name=f"sq0_{b}{c}", tag="sq",
                               bufs=3)
                eng("sq").tensor_tensor(out=sq, in0=pre[:, c, :],
                                        in1=pre[:, c, :], op=AL.mult)
                mm_stat(2, ob4[:, b, :], sq,
                        start=(b == 0 and c == 0), stop=(b == NB - 1 and c == 1),
                        tail_start=False,
                        tail_stop=(b == NB - 1 and c == 1))
            pre_t.append(pre)

        def ln_rows():
            """ps sections 1,2 -> (rstd, m) (4,SP) f32 rows; rB = rA*m."""
            m_r = read_stat(1, "m_r", scale=1.0 / D)
            v_r = read_stat(2, "v_r", scale=1.0 / D)
            msq = row4("msq")
            nc.scalar.activation(out=msq, in_=m_r, func=AF.Square)
            nc.vector.tensor_tensor(out=v_r, in0=v_r, in1=msq, op=AL.subtract)
            nc.scalar.activation(out=v_r, in_=v_r, func=AF.Sqrt,
                                 bias=eps4[:, 0:1])
            rstd = row4("rstd")
            nc.vector.reciprocal(out=rstd, in_=v_r)
            return rstd, m_r

        rstd, m_r = ln_rows()
        rA0 = row4("rA0", dt=F16)
        nc.vector.tensor_tensor(out=rA0, in0=rstd, in1=mask4, op=AL.mult)
        rB0 = row4("rB0", dt=F16)
        nc.vector.tensor_tensor(out=rB0, in0=rA0, in1=m_r, op=AL.mult)
        prefetch_apply_bc("i", rA0, rB0, None, mask4)
        for b in range(NB):
            aB = bounce_bcast(act0_in.ap()[b:b + 1, :], f"aBi{b}", "aBt")
            naB = napad(f"naBi{b}", _bcast_ap(nact0_in.ap()[b:b + 1, :]))
            bc_next[b] = bc_next[b] + (aB, naB, None)
        for b in range(NB):
            apply_ln(b, pre_t[b], bc_next[b][:4], seqT, True)
        pre_t = None

        # ================= main steps =======================================
        comp_t = [None] * NB

        lc_t = [None] * NB
        lc8_t = [None] * NB
        base_t = [None] * NB
        scan_t = [None] * NB
        inter_t = [None] * NB

        def phase_a(b, s):
            """apply + baseT + lc scan chain."""
            rAB, rBB, rCB, tpmB, aB, naB, ltpB = bc_next[b]
            if s > 0:
                apply_ln(b, comp_t[b], (rAB, rBB, rCB, tpmB), seqT, True)

            # ---- baseT = seqT + tf ----
            baseT = work.tile([128, DC, SP], CVT, name=f"baseT{b}",
                              tag="baseT", bufs=NB)
            if s == 0:
                for c in range(DC):
                    nc.vector.tensor_scalar(out=baseT[:, c, :],
                                            in0=seqT[b][:, c, :],
                                            scalar1=noc[:, c:c + 1],
                                            scalar2=None, op0=AL.add)
            else:
                for c in range(DC):
                    tfB = work.tile([128, SP], F16, name=f"tfB{b}{c}",
                                    tag="tfB", bufs=4)
                    nc.vector.tensor_scalar(out=tfB, in0=ltpB,
                                            scalar1=ymnc[:, c:c + 1],
                                            scalar2=noc[:, c:c + 1],
                                            op0=AL.mult, op1=AL.add)
                    nc.vector.tensor_tensor(out=baseT[:, c, :], in0=tfB,
                                            in1=seqT[b][:, c, :], op=AL.add)

            base_t[b] = baseT
            # ---- lc scan chain ----
            axB = ax_tile(f"axB{b}", "axB")
            for c in range(DC):
                eng("fill_seq").tensor_tensor(out=axB[:, c, 1:SP + 1], in0=aB,
                                              in1=seqT[b][:, c, :], op=AL.mult)
            lcT = work.tile([128, DC, SP], F16, name=f"lcT{b}", tag="lcT",
                            bufs=NB)
            for c in range(DC):
                scan_fwd(eng("scan_lc"), lcT[:, c, :], naB, axB[:, c])
            if F8_W1:
                lcT8 = work.tile([128, DC, SP], F8, name=f"lcT8_{b}",
                                 tag="lcT8", bufs=NB)
                for c in range(DC):
                    nc.vector.tensor_scalar(out=lcT8[:, c, :],
                                            in0=lcT[:, c, :], scalar1=1.0,
                                            scalar2=None, op0=AL.mult)
            else:
                lcT8 = lcT
            lc_t[b] = lcT
            lc8_t[b] = lcT8

        def phase_b(b, s):
            """w1 matmuls -> interT."""
            lcT8 = lc8_t[b]
            # ---- w1 -> gelu -> interT ----
            interT = work.tile([128, 8, SP], W2T, name=f"interT{b}",
                               tag="interT", bufs=NB)
            for hk in range(8):
                ps = psmm(f"ps_w1{b}{hk}")
                mmdr(ps, [(w1W8[0][:, :, hk * 128:(hk + 1) * 128], lcT8),
                          (w1W8[1][:, :, hk * 128:(hk + 1) * 128], seqT8[b])],
                     F8_W1)
                gelu_act(interT[:, hk, :], ps,
                         w1b_t[:, hk:hk + 1] if w1b_t is not None else None,
                         scale=w1_scale)
            inter_t[b] = interT

        def phase_c(b, s):
            """l1/r1/l2/r2 fills + scans."""
            _, _, _, _, aB, naB, _ = bc_next[b]
            baseT = base_t[b]
            axb = ax_tile(f"axb{b}", "axs")
            for c in range(DC):
                eng("fill_base").tensor_tensor(out=axb[:, c, 1:SP + 1], in0=aB,
                                               in1=baseT[:, c, :], op=AL.mult)
            l1T = work.tile([128, DC, SP], CVT, name=f"l1T{b}", tag="l1T", bufs=NB)
            r1T = work.tile([128, DC, SP], CVT, name=f"r1T{b}", tag="r1T", bufs=NB)
            for c in range(DC):
                scan_fwd(eng("scan_l1"), l1T[:, c, :], naB, axb[:, c])
                scan_bwd(eng("scan_r1"), r1T[:, c, :], naB, axb[:, c])
            ax2 = ax_tile(f"ax2{b}", "axs")
            for c in range(DC):
                eng("fill_l2").tensor_tensor(out=ax2[:, c, 1:SP + 1], in0=aB,
                                             in1=l1T[:, c, :], op=AL.mult)
            l2T = work.tile([128, DC, SP], CVT, name=f"l2T{b}", tag="l2T", bufs=NB)
            for c in range(DC):
                scan_fwd(eng("scan_l2"), l2T[:, c, :], naB, ax2[:, c])
            ax2b = ax_tile(f"ax2b{b}", "axs")
            for c in range(DC):
                eng("fill_r2").tensor_tensor(out=ax2b[:, c, 1:SP + 1], in0=aB,
                                             in1=r1T[:, c, :], op=AL.mult)
            r2T = work.tile([128, DC, SP], CVT, name=f"r2T{b}", tag="r2T", bufs=NB)
            for c in range(DC):
                scan_bwd(eng("scan_r2"), r2T[:, c, :], naB, ax2b[:, c])
            scan_t[b] = (l1T, r1T, l2T, r2T)

        def phase_dc(b, s):
            """conv -> gT -> tsc accumulate."""
            baseT = base_t[b]
            l1T, r1T, l2T, r2T = scan_t[b]
            # ---- conv -> gelu -> gT; tsc accumulate ----
            pieces = [(2, baseT), (1, l1T), (3, r1T), (0, l2T), (4, r2T)]
            for c in range(DC):
                ps = psmm(f"ps_cv{b}{c}")
                mmdr(ps, [(cvW8[w][:, :, c * 128:(c + 1) * 128], piece)
                          for (w, piece) in pieces], F8_CV)
                gT = work.tile([128, SP], F16, name=f"gT{b}{c}", tag="gT",
                               bufs=3)
                gelu_act(gT, ps,
                         convb_t[:, c:c + 1] if convb_t is not None else None,
                         scale=cv_scale)
                mm_stat(0, sc4[:, b, c, :], gT,
                        start=(b == 0 and c == 0), stop=(b == NB - 1 and c == 1),
                        tail_start=False, tail_stop=False, tail=False)

        def phase_dw(b, s):
            """w2 -> gates/parent -> comp -> LN stats."""
            lcT = lc_t[b]
            interT = inter_t[b]
            # ---- w2 -> gates/parent -> comp ----
            comp = work.tile([128, DC, SP], F16, name=f"comp{b}", tag="compT",
                             bufs=NB)
            parT = work.tile([128, DC, SP], F16, name=f"parT{b}", tag="gpar",
                             bufs=2)
            for g in [3, 0, 1, 2]:
                for c in range(DC):
                    cc = g * DC + c
                    ps = psmm(f"ps_w2{b}{cc}")
                    mmdr(ps, [(w2W8[p][:, :, cc * 128:(cc + 1) * 128],
                               interT[:, 2 * p:2 * p + 2, :]) for p in range(4)],
                         F8_W2)
                    bias = w2b_t[:, cc:cc + 1] if w2b_t is not None else 0.0
                    if g == 3:
                        nc.scalar.activation(out=parT[:, c, :], in_=ps,
                                             func=AF.Identity, bias=bias,
                                             scale=w2_scale)
                    else:
                        gate = work.tile([128, SP], F16, name=f"gate{b}",
                                         tag="gate", bufs=3)
                        nc.scalar.activation(out=gate, in_=ps, func=AF.Sigmoid,
                                             bias=bias, scale=w2_scale)
                        src = [lcT, seqT[b], parT][g]
                        if g == 0:
                            nc.vector.tensor_tensor(out=comp[:, c, :], in0=gate,
                                                    in1=src[:, c, :], op=AL.mult)
                        else:
                            gm = work.tile([128, SP], F16, name=f"gm{b}",
                                           tag="gmt", bufs=3)
                            eng("gm").tensor_tensor(out=gm, in0=gate,
                                                    in1=src[:, c, :], op=AL.mult)
                            nc.vector.tensor_tensor(out=comp[:, c, :],
                                                    in0=comp[:, c, :],
                                                    in1=gm, op=AL.add)
            comp_t[b] = comp

            # ---- LN stats of comp ----
            for c in range(DC):
                mm_stat(1, ob4[:, b, :], comp[:, c, :],
                        start=(b == 0 and c == 0), stop=(b == NB - 1 and c == 1),
                        tail_start=(b == 0 and c == 0), tail_stop=False)
            for c in range(DC):
                sq = work.tile([128, SP], F16, name=f"sq{b}{c}", tag="sq",
                               bufs=3)
                eng("sq").tensor_tensor(out=sq, in0=comp[:, c, :],
                                        in1=comp[:, c, :], op=AL.mult)
                mm_stat(2, ob4[:, b, :], sq,
                        start=(b == 0 and c == 0), stop=(b == NB - 1 and c == 1),
                        tail_start=False,
                        tail_stop=(b == NB - 1 and c == 1))

        def tail_tp(s):
            """tp/active rows; needs only the tsc stats (conv phase) -> runs
            concurrently with the w2 phase."""
            last = (s == n_steps - 1)
            if not last:
                asq = row4("asq")
                nc.vector.tensor_tensor(out=asq, in0=a4, in1=a4, op=AL.mult)

            # tp = selp * sigmoid(tsc): the reference's
            # tp = e^{t-mx}selp/(e^{t-mx}selp + e^{-mx} + EPS) equals this up
            # to O(EPS); scores are O(1) so no overflow concern.
            tsc = read_stat(0, "tsc", tail=False,
                            bias=scb_t[:, 0:1] if scb_t is not None else None)
            sig = row4("sig", dt=F16)
            nc.scalar.activation(out=sig, in_=tsc, func=AF.Sigmoid)
            tpp = work.tile([NB, SPP], F16, name="tpp", tag="rowP", bufs=3)
            nc.vector.memset(tpp[:, 0:SPP:SPP - 1], 0.0)
            tp = tpp[:, 1:SP + 1]
            nc.vector.tensor_tensor(out=tp, in0=sig, in1=selp4, op=AL.mult)
            tpm = row4("tpm", dt=F16)
            nc.vector.tensor_tensor(out=tpm, in0=tp, in1=mask4, op=AL.mult)
            rC = row4("rC", dt=F16)
            nc.vector.tensor_tensor(out=rC, in0=mask4, in1=tpm, op=AL.subtract)

            if last:
                return tpm, rC, None, None, None

            # active update: a_new = clip(a - a^2*u, 0, 1)*mask
            nap = work.tile([NB, SPP], F16, name="nap", tag="rowP", bufs=3)
            nc.vector.memset(nap[:, 0:SPP:SPP - 1], 0.0)
            nc.vector.tensor_scalar(out=nap[:, 1:SP + 1], in0=a4,
                                    scalar1=-1.0, scalar2=1.0,
                                    op0=AL.mult, op1=AL.add)
            u = row4("u")
            nc.vector.tensor_tensor_scan(
                out=u[:, ::-1], data0=nap[:, SPP - 1:1:-1],
                data1=tpp[:, SPP - 1:1:-1], initial=0.0,
                op0=AL.mult, op1=AL.add)
            nd = row4("nd")
            nc.vector.tensor_tensor(out=nd, in0=asq, in1=u, op=AL.mult)
            nc.vector.tensor_tensor(out=nd, in0=a4, in1=nd, op=AL.subtract)
            nc.vector.tensor_scalar(out=nd, in0=nd, scalar1=0.0,
                                    scalar2=1.0, op0=AL.max, op1=AL.min)
            nc.vector.tensor_tensor(out=a4, in0=nd, in1=mask4, op=AL.mult)
            a_f = row4("a_f", dt=F16)
            nc.vector.tensor_scalar(out=a_f, in0=a4, scalar1=1.0,
                                    scalar2=None, op0=AL.mult)
            na_f = row4("na_f", dt=F16)
            nc.vector.tensor_scalar(out=na_f, in0=a4, scalar1=-1.0,
                                    scalar2=1.0, op0=AL.mult, op1=AL.add)
            return tpm, rC, a_f, na_f, tp

        def tail_bc(s, rows):
            """row broadcasts at the step boundary: PE is idle there and the
            rows were computed during the w2 phase."""
            tpm, rC, a_f, na_f, tp = rows
            for b in range(NB):
                rCB = pe_bcast(rC, b, f"rCBs{s}{b}", "rCBt", "act")
                tpmB = (pe_bcast(tpm, b, f"tpmBs{s}{b}", "tpmBt", "act")
                        if lng_t is not None else None)
                if a_f is None:
                    bc_next[b] = (rCB, tpmB)
                else:
                    aB = pe_bcast(a_f, b, f"aBs{s}{b}", "aBt", "act")
                    naB = pe_bcast(na_f, b, f"naBs{s}{b}", None, "dve",
                                   pads=True)
                    ltpB = pe_bcast(tp, b, f"ltpBs{s}{b}", "ltpBt", "dve")
                    bc_next[b] = (rCB, tpmB, aB, naB, ltpB)

        def tail_ln(s, tpm):
            """LN gating rows; needs the w2-phase mean/var stats."""
            rstd, m_r = ln_rows()
            rA = row4("rA", dt=F16)
            nc.vector.tensor_tensor(out=rA, in0=tpm, in1=rstd, op=AL.mult)
            rB = row4("rB", dt=F16)
            nc.vector.tensor_tensor(out=rB, in0=rA, in1=m_r, op=AL.mult)
            for b in range(NB):
                rAB = pe_bcast(rA, b, f"rABs{s}{b}", "rABt", "dve")
                rBB = pe_bcast(rB, b, f"rBBs{s}{b}", "rBBt", "act")
                bc_next[b] = (rAB, rBB) + bc_next[b]

        for s in range(n_steps):
            for b in range(NB):
                phase_a(b, s)
            for b in range(NB):
                phase_b(b, s)
            for b in range(NB):
                phase_c(b, s)
            for b in range(NB):
                phase_dc(b, s)
            rows = tail_tp(s)
            for b in range(NB):
                phase_dw(b, s)
            tail_bc(s, rows)
            tail_ln(s, rows[0])

        # ---------------- final apply (f32 out) + DMA ----------------------
        outF = [work.tile([128, DC, SP], F32, name=f"outF{b}", tag="outF",
                          bufs=2) for b in range(NB)]

        def out_dma(b, c, dst):
            nc.sync.dma_start(out=out_dram.ap()[b, c], in_=dst[:, c, 0:S2])

        for b in range(NB):
            apply_ln(b, comp_t[b], bc_next[b][:4], outF, False,
                     per_chunk_dma=out_dma)
    return nc


def _host_prep(inputs):
    f32 = np.float32
    f16 = np.float16
    f8 = ml_dtypes.float8_e4m3
    seq = np.asarray(inputs["sequence"], f32)
    im = np.asarray(inputs["input_mask"], f32)
    START = np.asarray(inputs["START"], f32)
    END = np.asarray(inputs["END"], f32)
    yes_t = np.asarray(inputs["yes_t"], f32).reshape(-1)
    no_t = np.asarray(inputs["no_t"], f32).reshape(-1)
    N, S, Dd = seq.shape
    assert (N, S, Dd) == (32, 512, 256), (N, S, Dd)

    ones = np.ones((N, 1, 1), f32)
    zeros = np.zeros((N, 1, 1), f32)
    mask0 = np.concatenate([ones, im], 1)
    mask_no_end = np.concatenate([mask0, zeros], 1)
    mask_yes_end = np.concatenate([ones, mask0], 1)
    END_mask = mask_yes_end - mask_no_end
    seqA = np.concatenate([np.broadcast_to(START, (N, 1, Dd)), seq,
                           np.zeros((N, 1, Dd), f32)], 1)
    seqA = (END_mask * END + (1.0 - END_mask) * seqA).astype(f32)
    mask = mask_yes_end
    mask_no_start = np.concatenate([zeros, mask[:, 1:]], 1)
    last_tok = np.concatenate([END_mask[:, 1:], zeros], 1)
    selp = (mask_no_start * mask_no_end * (1.0 - last_tok)).astype(f32)

    seqT = np.zeros((N, DC, 128, SP), f32)
    for c in range(DC):
        seqT[:, c, :, :S2] = seqA[:, :, c * 128:(c + 1) * 128].transpose(0, 2, 1)
    maskP = np.zeros((N, SP), f32)
    maskP[:, :S2] = mask[:, :, 0]
    selpP = np.zeros((N, SP), f32)
    selpP[:, :S2] = selp[:, :, 0]
    actP = maskP.copy()
    nactP = (1.0 - actP).astype(f32)

    def chunk_col(v, nch):
        return np.ascontiguousarray(np.asarray(v, f32).reshape(nch, 128).T)

    scW = np.asarray(inputs["scW"], f32).reshape(-1)
    sc4 = np.zeros((128, NB, DC, 4), f32)
    for b in range(NB):
        for c in range(DC):
            sc4[:, b, c, b] = scW[c * 128:(c + 1) * 128]
    ob4 = np.zeros((128, NB, 4), f32)
    for b in range(NB):
        ob4[:, b, b] = 1.0
    bsel = np.zeros((4, NB, 128), f32)
    for b in range(NB):
        bsel[b, b, :] = 1.0

    def wconv(name, use_f8):
        w = np.asarray(inputs[name], f32)
        return (w * WSCALE).astype(f8) if use_f8 else w.astype(f16)

    host = {
        "seqT": seqT.astype(f16),
        "mask": maskP.astype(f16), "selp": selpP.astype(f16),
        "act0": actP.astype(f16), "act0f": actP,
        "nact0": nactP.astype(f16),
        "itW": np.asarray(inputs["itW"], f32).astype(f16),
        "convW": wconv("convW", F8_CV),
        "w1W": wconv("w1W", F8_W1),
        "w2W": wconv("w2W", F8_W2),
        "sc4": sc4.astype(f16),
        "ob4": ob4.astype(f16),
        "bsel": bsel.astype(f16),
        "noc": chunk_col(no_t, DC),
        "ymnc": chunk_col(yes_t - no_t, DC),
    }
    flags = {
        "itbc": bool(np.any(np.asarray(inputs["itb"]))),
        "convbc": bool(np.any(np.asarray(inputs["convb"]))),
        "w1bc": bool(np.any(np.asarray(inputs["w1b"]))),
        "w2bc": bool(np.any(np.asarray(inputs["w2b"]))),
        "scbc": bool(np.any(np.asarray(inputs["scb"]))),
        "lngc": bool(np.any(np.asarray(inputs["lnb"])))
        or bool(np.any(np.asarray(inputs["lng"]) != 1.0)),
    }
    flags["lnbc"] = flags["lngc"]
    if flags["itbc"]:
        host["itbc"] = chunk_col(inputs["itb"], DC)
    if flags["convbc"]:
        host["convbc"] = chunk_col(inputs["convb"], DC)
    if flags["w1bc"]:
        host["w1bc"] = chunk_col(inputs["w1b"], 8)
    if flags["w2bc"]:
        host["w2bc"] = chunk_col(inputs["w2b"], 8)
    if flags["scbc"]:
        host["scbc"] = np.broadcast_to(
            np.asarray(inputs["scb"], f32).reshape(1, 1), (4, 1)).copy()
    if flags["lngc"]:
        host["lngc"] = chunk_col(inputs["lng"], DC)
        host["lnbc"] = chunk_col(inputs["lnb"], DC)
    return host, flags


_PROG_CACHE = {}


def kernel(**inputs):
    global LAST_EXEC_NS, LAST_RES
    n_steps = int(inputs["n_steps"])
    host, flags = _host_prep(inputs)

    key = (n_steps, tuple(sorted(flags.items())), MM_DT, W2_DT, SIM, GP_LVL,
           F8_W1, F8_W2, F8_CV)
    if key not in _PROG_CACHE:
        _PROG_CACHE[key] = _build_program(n_steps, flags)
    nc = _PROG_CACHE[key]

    per_batch = {"seqT", "mask", "selp", "act0", "act0f", "nact0"}
    in_maps = []
    for k in range(NCORES):
        m = {}
        for name, arr in host.items():
            if name in per_batch:
                m[name] = np.ascontiguousarray(arr[k * NB:(k + 1) * NB])
            else:
                m[name] = arr
        in_maps.append(m)

    if SIM:
        from concourse.bass_interp import CoreSim
        results = []
        for k in range(int(os.environ.get("CRVNN_SIM_CORES", "1"))):
            sim = CoreSim(nc)
            for name, v in in_maps[k].items():
                sim.tensor(name)[:] = v
            sim.simulate()
            results.append(np.array(sim.tensor("out")))
    else:
        from concourse.bass_utils import run_bass_kernel_spmd
        if not getattr(nc, "_waitfix_done", False):
            _split_multiwaits(nc)
            nc._waitfix_done = True
        res = run_bass_kernel_spmd(nc, in_maps, list(range(NCORES)), trace=TRACE)
        LAST_EXEC_NS = res.exec_time_ns
        LAST_RES = res
        results = [res.results[k]["out"] for k in range(NCORES)]

    full = np.zeros((32, S2, D), np.float32)
    for k, o in enumerate(results):
        for b in range(NB):
            for c in range(DC):
                full[k * NB + b, :, c * 128:(c + 1) * 128] = \
                    np.asarray(o[b, c], np.float32).T
    return full


# revision 31
# speedup vs baseline: 1.4903x; 1.0447x over previous
"""CRvNN forward kernel for 8x Trainium2 NeuronCores (Bass/Tile), v3.

Strategy
--------
Pure data parallelism: batch 32 -> 4 per core; params replicated; no
collectives.  State is TRANSPOSED (D=256 on partitions as 2x128 chunks,
position on the free axis, padded 514 -> 516).  The S^2 neighbor-prob
matrices are first-order linear recurrences evaluated as tensor_tensor_
scans; they are never materialized.

v3 design (vs the 965us v1 baseline):
- w2 (1024x1024, 60% of PE work) runs in fp8e4 + MatmulPerfMode.DoubleRow:
  one instruction contracts a 256-row pair at 0.5 cyc/col (4x f32r).  The
  w2 weights are host-prescaled by 64 (fp8 range) and 1/64 is folded into
  the PSUM-read activation scale; interT is written fp8 by the w1 gelu.
- Everything else lives in FLOAT16: same 10-bit mantissa as f32r (so
  near-f32r accuracy), but 2-byte, so DVE tensor_tensor runs 2x (327ns
  vs 594 for a (128,516) op).  fp16 range is safe: all tensors here are
  bounded by ~1e3.  w1/conv matmuls run plain fp16 (1 cyc/col, same as
  f32r, zero rhs-quantization error).
- All (1,516) row math (transition probs, active update, LN stats) is
  batched across the 4 local batches as (4,516) tiles.  LN mean/var and
  the score matvec accumulate into bank-aligned PSUM sections
  (4,3,512)+(4,12) via one-hot lhsT selectors, so partition b receives
  batch b directly from PE.  tp uses the algebraic identity
  tp = selp * sigmoid(tsc) (exact up to the reference's 1e-9 EPS), which
  cuts the serial tail chain from ~12 to ~3 ops.
- Step-boundary latency: the tail DMAs its row groups to DRAM and
  immediately issues ALL next-step partition-broadcast loads, so the
  DRAM round trip overlaps the remaining tail math and the next step's
  applies start as soon as their rows land.
- Engine balancing: the l1/r1/l2/r2 scans and some fills run on Pool
  (gpsimd); the rest of the elementwise work stays on DVE at fp16 rates.

This walrus build supports only ONE sync wait per instruction; a
post-scheduling pass splits multi-wait instructions into single-wait NOP
chains.
"""
import os
import sys
from contextlib import ExitStack

import numpy as np

sys.path.insert(0, "/opt/trn_rl_repo")

import ml_dtypes
import bass_rust
import concourse.bass as bass
import concourse.mybir as mybir
from concourse.tile import TileContext

F32 = mybir.dt.float32
F16 = mybir.dt.float16
BF16 = mybir.dt.bfloat16
F8 = mybir.dt.float8e4
AL = mybir.AluOpType
AF = mybir.ActivationFunctionType
PM = mybir.MatmulPerfMode

NCORES = 8
NB = 4            # batch per core
D = 256
DC = 2            # D chunks of 128
S2 = 514
SP = 516          # padded sequence length
SPP = SP + 2      # scan-input tiles have leading+trailing zero pad columns
H = 1024
WIN = 5
EPS = 1e-9
WSCALE = 64.0     # fp8 weight prescale (folded back via activation scale)

SIM = os.environ.get("CRVNN_SIM", "0") == "1"
TRACE = os.environ.get("CRVNN_TRACE", "0") == "1"
# compat attrs (test.py uses these in its program-cache key)
MM_DT = os.environ.get("CRVNN_MMDT", "f8")
W2_DT = os.environ.get("CRVNN_W2DT", "f8")
GP_LVL = int(os.environ.get("CRVNN_GP", "1"))
# per-matmul-group dtype: "f8" = fp8e4 + DoubleRow; anything else = fp16
F8_W1 = os.environ.get("CRVNN_F8W1", "f16") == "f8"
F8_W2 = os.environ.get("CRVNN_F8W2", "f8") == "f8"
F8_CV = os.environ.get("CRVNN_F8CV", "f8") == "f8"

NSPLITS = [(0, 512), (512, SP - 512)]

LAST_EXEC_NS = None
LAST_RES = None

# engine assignment knobs: 'v' = DVE, 'g' = Pool/gpsimd
ENG = {
    "fill_seq": "v",
    "fill_base": "g",
    "fill_l2": "g",
    "fill_r2": "g",
    "scan_lc": "v",
    "scan_l1": "v",
    "scan_r1": "v",
    "scan_l2": "v",
    "scan_r2": "v",
    "gm": "g",
    "t2": "g",
    "sq": "g",
}


# --------------------------------------------------------------------------
# post-scheduling fixup: split multi-wait instructions into 1-wait NOP chains
# --------------------------------------------------------------------------
def _split_multiwaits(nc):
    counter = [0]

    def mk_nop(engine, wait):
        counter[0] += 1
        n = bass_rust.InstNoOp(name=f"WFIX-{counter[0]}", ins=[], outs=[])
        n.engine = engine
        n.sync_info = bass_rust.SyncInfo(on_wait=[wait], on_update=[])
        return n

    total = 0
    for f in nc.m.functions:
        for bb in f.blocks:
            out = []
            changed = False
            for inst in list(bb.instructions):
                si = inst.sync_info
                waits = list(si.on_wait) if (si is not None and si.on_wait) else []
                if len(waits) > 1:
                    for w in waits[:-1]:
                        out.append(mk_nop(inst.engine, w))
                    inst.sync_info = bass_rust.SyncInfo(
                        on_wait=[waits[-1]],
                        on_update=list(si.on_update) if si.on_update else [])
                    changed = True
                    total += 1
                out.append(inst)
            if changed:
                bb.instructions = out
    return total


def _bcast_ap(drow):
    """DRAM row AP (1, n) -> partition-broadcast AP (128, n)."""
    return bass.AP(tensor=drow.tensor, offset=drow.offset,
                   ap=[[0, 128]] + drow.ap[1:])


def _build_program(n_steps, flags):
    nc = bass.Bass()

    W1T = F8 if F8_W1 else F16
    W2T = F8 if F8_W2 else F16
    CVT = F8 if F8_CV else F16
    w1_scale = 1.0 / WSCALE if F8_W1 else 1.0
    w2_scale = 1.0 / WSCALE if F8_W2 else 1.0
    cv_scale = 1.0 / WSCALE if F8_CV else 1.0

    seqT_in = nc.declare_dram_parameter("seqT", [NB, DC, 128, SP], F16, isOutput=False)
    mask_in = nc.declare_dram_parameter("mask", [NB, SP], F16, isOutput=False)
    selp_in = nc.declare_dram_parameter("selp", [NB, SP], F16, isOutput=False)
    act0_in = nc.declare_dram_parameter("act0", [NB, SP], F16, isOutput=False)
    act0f_in = nc.declare_dram_parameter("act0f", [NB, SP], F32, isOutput=False)
    nact0_in = nc.declare_dram_parameter("nact0", [NB, SP], F16, isOutput=False)
    itW_in = nc.declare_dram_parameter("itW", [D, D], F16, isOutput=False)
    convW_in = nc.declare_dram_parameter("convW", [WIN * D, D], CVT, isOutput=False)
    w1W_in = nc.declare_dram_parameter("w1W", [2 * D, H], W1T, isOutput=False)
    w2W_in = nc.declare_dram_parameter("w2W", [H, 4 * D], W2T, isOutput=False)
    sc4_in = nc.declare_dram_parameter("sc4", [128, NB, DC, 4], F16, isOutput=False)
    ob4_in = nc.declare_dram_parameter("ob4", [128, NB, 4], F16, isOutput=False)
    bsel_in = nc.declare_dram_parameter("bsel", [4, NB, 128], F16, isOutput=False)
    noc_in = nc.declare_dram_parameter("noc", [128, DC], F32, isOutput=False)
    ymn_in = nc.declare_dram_parameter("ymnc", [128, DC], F32, isOutput=False)
    opt_in = {}
    for nm, shape in [("itbc", [128, DC]), ("convbc", [128, DC]),
                      ("w1bc", [128, 8]), ("w2bc", [128, 8]), ("scbc", [4, 1]),
                      ("lngc", [128, DC]), ("lnbc", [128, DC])]:
        if flags.get(nm):
            opt_in[nm] = nc.declare_dram_parameter(nm, shape, F32, isOutput=False)
    out_dram = nc.declare_dram_parameter("out", [NB, DC, 128, S2], F32, isOutput=True)

    with TileContext(nc) as tc, ExitStack() as ctx:
        wpool = ctx.enter_context(tc.tile_pool(name="wpool", bufs=1))
        state = ctx.enter_context(tc.tile_pool(name="state", bufs=1))
        work = ctx.enter_context(tc.tile_pool(name="work", bufs=1))
        psum = ctx.enter_context(tc.tile_pool(name="psum", bufs=1, space="PSUM"))
        dram = ctx.enter_context(tc.tile_pool(name="dramp", bufs=1, space="DRAM"))

        V = nc.vector
        G = nc.gpsimd

        def eng(key):
            return G if ENG[key] == "g" else V

        # ---------------- weights -> SBUF ----------------------------------
        # pair tiles: (128, 2, M); [:, i, :] = rows [p*256 + i*128 : +128]
        # (host already converted to the matmul dtype)
        def load_pairs(name, dram_p, n_pairs, M, dt):
            tiles = []
            for p in range(n_pairs):
                t = wpool.tile([128, 2, M], dt, name=f"{name}{p}")
                nc.sync.dma_start(
                    out=t,
                    in_=dram_p.ap()[p * 256:(p + 1) * 256, :].rearrange(
                        "(two q) m -> q two m", two=2))
                tiles.append(t)
            return tiles

        w1W8 = load_pairs("w1W8", w1W_in, 2, H, W1T)
        w2W8 = load_pairs("w2W8", w2W_in, 4, 4 * D, W2T)
        cvW8 = load_pairs("cvW8", convW_in, 5, D, CVT)

        itW_t = wpool.tile([128, 2, D], F16, name="itWt")
        nc.sync.dma_start(out=itW_t,
                          in_=itW_in.ap().rearrange("(two q) m -> q two m", two=2))
        sc4 = wpool.tile([128, NB, DC, 4], F16, name="sc4t")
        nc.sync.dma_start(out=sc4, in_=sc4_in.ap())
        ob4 = wpool.tile([128, NB, 4], F16, name="ob4t")
        nc.sync.dma_start(out=ob4, in_=ob4_in.ap())
        bsel = wpool.tile([4, NB, 128], F16, name="bselt")
        nc.sync.dma_start(out=bsel, in_=bsel_in.ap())
        noc = wpool.tile([128, DC], F32, name="noct")
        nc.sync.dma_start(out=noc, in_=noc_in.ap())
        ymnc = wpool.tile([128, DC], F32, name="ymnct")
        nc.sync.dma_start(out=ymnc, in_=ymn_in.ap())
        eps4 = wpool.tile([4, 1], F32, name="eps4")
        nc.vector.memset(eps4, 1e-5)

        def load_opt(nm, shape):
            if nm not in opt_in:
                return None
            t = wpool.tile(shape, F32, name=f"{nm}_t")
            nc.sync.dma_start(out=t, in_=opt_in[nm].ap())
            return t

        itb_t = load_opt("itbc", [128, DC])
        convb_t = load_opt("convbc", [128, DC])
        w1b_t = load_opt("w1bc", [128, 8])
        w2b_t = load_opt("w2bc", [128, 8])
        scb_t = load_opt("scbc", [4, 1])
        lng_t = load_opt("lngc", [128, DC])
        lnb_t = load_opt("lnbc", [128, DC])

        # ---------------- persistent state ---------------------------------
        seqT = [state.tile([128, DC, SP], F16, name=f"seqT{b}") for b in range(NB)]
        if F8_W1:
            seqT8 = [state.tile([128, DC, SP], F8, name=f"seqT8_{b}")
                     for b in range(NB)]
        else:
            seqT8 = seqT
        a4 = state.tile([NB, SP], F32, name="a4")
        nc.sync.dma_start(out=a4, in_=act0f_in.ap())
        mask4 = state.tile([NB, SP], F16, name="mask4")
        nc.sync.dma_start(out=mask4, in_=mask_in.ap())
        selp4 = state.tile([NB, SP], F16, name="selp4")
        nc.sync.dma_start(out=selp4, in_=selp_in.ap())

        # PSUM: matmul tiles + batched-stat sections
        def psmm(name):
            return psum.tile([128, SP], F32, name=name, tag="psmm", bufs=2)

        ps_big = psum.tile([4, 3, 512], F32, name="ps_big", tag="psbig", bufs=1)
        ps_tail = psum.tile([4, 12], F32, name="ps_tail", tag="pstail", bufs=1)

        def row4(name, dt=F32, bufs=8):
            return work.tile([NB, SP], dt, name=name, tag="row4", bufs=bufs)

        def bc_tile(name, tag, bufs):
            return work.tile([128, SP], F16, name=name, tag=tag, bufs=bufs)

        def bounce_bcast(drow_b, name, tag, bufs=5):
            """(1,SP) slice of a DRAM (4,SP) tile -> (128,SP) bcast tile."""
            t = bc_tile(name, tag=tag, bufs=bufs)
            nc.sync.dma_start(out=t, in_=_bcast_ap(drow_b))
            return t

        def napad(name, src_ap):
            """(128, SPP) bcast tile with zero pads at cols 0, SPP-1."""
            t = work.tile([128, SPP], F16, name=name, tag="nabP", bufs=5)
            nc.vector.memset(t[:, 0:SPP:SPP - 1], 0.0)
            nc.sync.dma_start(out=t[:, 1:SP + 1], in_=src_ap)
            return t

        def pe_bcast(row, b, name, tag, copy_eng, pads=False, bufs=5):
            """Broadcast row b of a (4,SP) SBUF tile to (128,SP) via PE:
            psum[p,i] = sum_q bsel[q,b,p]*row[q,i] = row[b,i], then one
            engine copy PSUM->SBUF.  No DRAM round trip."""
            ps = psmm(f"bc_{name}")
            for (o, s) in NSPLITS:
                nc.tensor.matmul(ps[:, o:o + s], bsel[:, b, :],
                                 row[:, o:o + s], start=True, stop=True)
            if pads:
                t = work.tile([128, SPP], F16, name=name, tag="nabP",
                              bufs=bufs)
                nc.vector.memset(t[:, 0:SPP:SPP - 1], 0.0)
                dst = t[:, 1:SP + 1]
            else:
                t = bc_tile(name, tag=tag, bufs=bufs)
                dst = t
            if copy_eng == "act":
                nc.scalar.activation(out=dst, in_=ps, func=AF.Copy)
            elif copy_eng == "pool":
                nc.gpsimd.tensor_scalar(out=dst, in0=ps, scalar1=1.0,
                                        scalar2=None, op0=AL.mult)
            else:
                nc.vector.tensor_scalar(out=dst, in0=ps, scalar1=1.0,
                                        scalar2=None, op0=AL.mult)
            return t

        def ax_tile(name, tag):
            t = work.tile([128, DC, SPP], F16, name=name, tag=tag,
                          bufs=(4 if tag == "axs" else 3))
            for c in range(DC):
                nc.vector.memset(t[:, c, 0:SPP:SPP - 1], 0.0)
            return t

        def scan_fwd(e, out_c, nap, datap):
            """out[i] = data[i-1] + na[i-1]*out[i-1]; data pad supplies z0=0."""
            e.tensor_tensor_scan(
                out=out_c, data0=nap[:, 0:SP], data1=datap[:, 0:SP],
                initial=0.0, op0=AL.mult, op1=AL.add)

        def scan_bwd(e, out_c, nap, datap):
            e.tensor_tensor_scan(
                out=out_c[:, ::-1], data0=nap[:, SPP - 1:1:-1],
                data1=datap[:, SPP - 1:1:-1], initial=0.0,
                op0=AL.mult, op1=AL.add)

        def gelu_act(out, in_, bias, scale=1.0):
            b = bias if bias is not None else 0.0
            if SIM:
                n = out.shape[-1]
                x2 = work.tile([out.shape[0], n], F32, name="gx2",
                               tag="gelu_tmp", bufs=2)
                nc.scalar.activation(out=x2, in_=in_, func=AF.Square, bias=b,
                                     scale=scale)
                nc.vector.tensor_scalar(out=x2, in0=x2, scalar1=0.044715,
                                        scalar2=1.0, op0=AL.mult, op1=AL.add)
                u = work.tile([out.shape[0], n], F32, name="gu",
                              tag="gelu_tmp2", bufs=2)
                nc.scalar.activation(out=u, in_=in_, func=AF.Identity, bias=b,
                                     scale=scale)
                nc.vector.tensor_tensor(out=x2, in0=x2, in1=u, op=AL.mult)
                nc.scalar.activation(out=x2, in_=x2, func=AF.Tanh,
                                     scale=0.7978845608028654)
                nc.vector.tensor_scalar(out=x2, in0=x2, scalar1=1.0,
                                        scalar2=0.5, op0=AL.add, op1=AL.mult)
                nc.vector.tensor_tensor(out=out, in0=x2, in1=u, op=AL.mult)
            else:
                nc.scalar.activation(out=out, in_=in_, func=AF.Gelu_apprx_tanh,
                                     bias=b, scale=scale)

        # matmul helper: lhsT pair tiles, rhs (128, 2, s) slices
        def mmdr(ps_ap, pairs, f8, nsl=NSPLITS):
            K = len(pairs)
            for (o, s) in nsl:
                for k, (lhsT, rhs) in enumerate(pairs):
                    if f8:
                        nc.tensor.matmul(ps_ap[:, o:o + s], lhsT,
                                         rhs[:, :, o:o + s],
                                         start=(k == 0), stop=(k == K - 1),
                                         perf_mode=PM.DoubleRow)
                    else:
                        for i in range(2):
                            nc.tensor.matmul(ps_ap[:, o:o + s], lhsT[:, i, :],
                                             rhs[:, i, o:o + s],
                                             start=(k == 0 and i == 0),
                                             stop=(k == K - 1 and i == 1))

        # batched-stat matmul into ps_big/ps_tail section t.  ps_tail's three
        # sections share one PSUM bank = one accumulation group per step.
        def mm_stat(t, lhsT, rhs, start, stop, tail_start, tail_stop,
                    tail=True):
            nc.tensor.matmul(ps_big[:, t, :], lhsT, rhs[:, 0:512],
                             start=start, stop=stop)
            if tail:
                nc.tensor.matmul(ps_tail[:, 4 * t:4 * t + 4], lhsT,
                                 rhs[:, 512:SP],
                                 start=tail_start, stop=tail_stop)

        def read_stat(t, name, bias=None, scale=1.0, dt=F32, tail=True):
            r = row4(name, dt=dt)
            func = AF.Copy if bias is None else AF.Identity
            b = 0.0 if bias is None else bias
            nc.scalar.activation(out=r[:, 0:512], in_=ps_big[:, t, :],
                                 func=func, bias=b, scale=scale)
            if tail:
                nc.scalar.activation(out=r[:, 512:SP],
                                     in_=ps_tail[:, 4 * t:4 * t + 4],
                                     func=func, bias=b, scale=scale)
            else:
                # tsc cols >= 512 are always selp-masked to zero downstream
                nc.vector.memset(r[:, 512:SP], 0.0)
            return r

        # ------------------------------------------------------------------
        # apply: seq_new = rA*pre - rB [ *lng + gate*lnb ] (+ rC*seq_old)
        # ------------------------------------------------------------------
        def apply_ln(b, pre, bc, dst, mk_shadow, per_chunk_dma=None,
                     t2_pre=None):
            rAB, rBB, rCB, gateB = bc
            for c in range(DC):
                t1 = work.tile([128, SP], F16, name="t1g", tag="t1g", bufs=4)
                nc.vector.tensor_tensor(out=t1, in0=rAB, in1=pre[:, c, :],
                                        op=AL.mult)
                nc.vector.tensor_tensor(out=t1, in0=t1, in1=rBB, op=AL.subtract)
                if lng_t is not None:
                    nc.vector.tensor_scalar(out=t1, in0=t1,
                                            scalar1=lng_t[:, c:c + 1],
                                            scalar2=None, op0=AL.mult)
                    nc.vector.scalar_tensor_tensor(
                        out=t1, in0=gateB, scalar=lnb_t[:, c:c + 1], in1=t1,
                        op0=AL.mult, op1=AL.add)
                if rCB is None:
                    nc.vector.tensor_copy(out=dst[b][:, c, :], in_=t1)
                else:
                    if t2_pre is not None:
                        t2 = t2_pre[c]
                    else:
                        t2 = work.tile([128, SP], F16, name="t2g", tag="t2g",
                                       bufs=4)
                        eng("t2").tensor_tensor(out=t2, in0=rCB,
                                                in1=seqT[b][:, c, :],
                                                op=AL.mult)
                    nc.vector.tensor_tensor(out=dst[b][:, c, :], in0=t1,
                                            in1=t2, op=AL.add)
                if mk_shadow and F8_W1:
                    nc.vector.tensor_scalar(out=seqT8[b][:, c, :],
                                            in0=seqT[b][:, c, :],
                                            scalar1=1.0, scalar2=None,
                                            op0=AL.mult)
                if per_chunk_dma is not None:
                    per_chunk_dma(b, c, dst[b])

        # prefetched broadcast tiles for the next step, keyed per batch
        bc_next = {}
        t2_pre = {}

        def prefetch_apply_bc(suffix, rA, rB, rC, tpm):
            for b in range(NB):
                rAB = pe_bcast(rA, b, f"rAB{suffix}{b}", "rABt", "act")
                rBB = pe_bcast(rB, b, f"rBB{suffix}{b}", "rBBt", "act")
                rCB = (pe_bcast(rC, b, f"rCB{suffix}{b}", "rCBt", "dve")
                       if rC is not None else None)
                tpmB = None
                if lng_t is not None:
                    tpmB = pe_bcast(tpm, b, f"tpmB{suffix}{b}", "tpmBt", "act")
                bc_next[b] = (rAB, rBB, rCB, tpmB)

        def prefetch_row_bc(suffix, a_f, na_f, ltp):
            for b in range(NB):
                aB = pe_bcast(a_f, b, f"aB{suffix}{b}", "aBt", "act")
                naB = pe_bcast(na_f, b, f"naB{suffix}{b}", None, "pool",
                               pads=True)
                ltpB = (pe_bcast(ltp, b, f"ltpB{suffix}{b}", "ltpBt", "dve")
                        if ltp is not None else None)
                bc_next[b] = bc_next[b] + (aB, naB, ltpB)

        # ================= initial transform ================================
        pre_t = []
        for b in range(NB):
            sA = work.tile([128, DC, SP], F16, name=f"sA{b}", tag="sA", bufs=2)
            nc.sync.dma_start(out=sA,
                              in_=seqT_in.ap()[b].rearrange("c p i -> p c i"))
            pre = work.tile([128, DC, SP], F16, name=f"pre{b}", tag="compT",
                            bufs=NB)
            for c in range(DC):
                ps = psmm(f"ps_pre{b}{c}")
                for (o, s) in NSPLITS:
                    for k in range(2):
                        nc.tensor.matmul(ps[:, o:o + s],
                                         itW_t[:, k, c * 128:(c + 1) * 128],
                                         sA[:, k, o:o + s],
                                         start=(k == 0), stop=(k == 1))
                if itb_t is not None:
                    nc.scalar.activation(out=pre[:, c, :], in_=ps,
                                         func=AF.Identity,
                                         bias=itb_t[:, c:c + 1])
                else:
                    nc.scalar.activation(out=pre[:, c, :], in_=ps, func=AF.Copy)
            for c in range(DC):
                mm_stat(1, ob4[:, b, :], pre[:, c, :],
                        start=(b == 0 and c == 0), stop=(b == NB - 1 and c == 1),
                        tail_start=(b == 0 and c == 0), tail_stop=False)
            for c in range(DC):
                sq = work.tile([128, SP], F16, name=f"sq0_{b}{c}", tag="sq",
                               bufs=4)
                eng("sq").tensor_tensor(out=sq, in0=pre[:, c, :],
                                        in1=pre[:, c, :], op=AL.mult)
                mm_stat(2, ob4[:, b, :], sq,
                        start=(b == 0 and c == 0), stop=(b == NB - 1 and c == 1),
                        tail_start=False,
                        tail_stop=(b == NB - 1 and c == 1))
            pre_t.append(pre)

        def ln_rows():
            """ps sections 1,2 -> (rstd, m) (4,SP) f32 rows; rB = rA*m."""
            m_r = read_stat(1, "m_r", scale=1.0 / D)
            v_r = read_stat(2, "v_r", scale=1.0 / D)
            msq = row4("msq")
            nc.scalar.activation(out=msq, in_=m_r, func=AF.Square)
            nc.vector.tensor_tensor(out=v_r, in0=v_r, in1=msq, op=AL.subtract)
            nc.scalar.activation(out=v_r, in_=v_r, func=AF.Sqrt,
                                 bias=eps4[:, 0:1])
            rstd = row4("rstd")
            nc.vector.reciprocal(out=rstd, in_=v_r)
            return rstd, m_r

        rstd, m_r = ln_rows()
        rA0 = row4("rA0", dt=F16)
        nc.vector.tensor_tensor(out=rA0, in0=rstd, in1=mask4, op=AL.mult)
        rB0 = row4("rB0", dt=F16)
        nc.vector.tensor_tensor(out=rB0, in0=rA0, in1=m_r, op=AL.mult)
        prefetch_apply_bc("i", rA0, rB0, None, mask4)
        for b in range(NB):
            aB = bounce_bcast(act0_in.ap()[b:b + 1, :], f"aBi{b}", "aBt")
            naB = napad(f"naBi{b}", _bcast_ap(nact0_in.ap()[b:b + 1, :]))
            bc_next[b] = bc_next[b] + (aB, naB, None)
        for b in range(NB):
            apply_ln(b, pre_t[b], bc_next[b][:4], seqT, True)
        pre_t = None

        # ================= main steps =======================================
        comp_t = [None] * NB

        lc_t = [None] * NB
        lc8_t = [None] * NB
        base_t = [None] * NB
        scan_t = [None] * NB
        inter_t = [None] * NB

        def phase_a(b, s):
            """apply + baseT + lc scan chain."""
            rAB, rBB, rCB, tpmB, aB, naB, ltpB = bc_next[b]
            if s > 0:
                apply_ln(b, comp_t[b], (rAB, rBB, rCB, tpmB), seqT, True,
                         t2_pre=t2_pre.get(b))

            # ---- baseT = seqT + tf ----
            baseT = work.tile([128, DC, SP], CVT, name=f"baseT{b}",
                              tag="baseT", bufs=NB)
            if s == 0:
                for c in range(DC):
                    nc.vector.tensor_scalar(out=baseT[:, c, :],
                                            in0=seqT[b][:, c, :],
                                            scalar1=noc[:, c:c + 1],
                                            scalar2=None, op0=AL.add)
            else:
                for c in range(DC):
                    tfB = work.tile([128, SP], F16, name=f"tfB{b}{c}",
                                    tag="tfB", bufs=4)
                    nc.vector.tensor_scalar(out=tfB, in0=ltpB,
                                            scalar1=ymnc[:, c:c + 1],
                                            scalar2=noc[:, c:c + 1],
                                            op0=AL.mult, op1=AL.add)
                    nc.vector.tensor_tensor(out=baseT[:, c, :], in0=tfB,
                                            in1=seqT[b][:, c, :], op=AL.add)

            base_t[b] = baseT
            # ---- lc scan chain ----
            axB = ax_tile(f"axB{b}", "axB")
            for c in range(DC):
                eng("fill_seq").tensor_tensor(out=axB[:, c, 1:SP + 1], in0=aB,
                                              in1=seqT[b][:, c, :], op=AL.mult)
            lcT = work.tile([128, DC, SP], F16, name=f"lcT{b}", tag="lcT",
                            bufs=NB)
            for c in range(DC):
                scan_fwd(eng("scan_lc"), lcT[:, c, :], naB, axB[:, c])
            if F8_W1:
                lcT8 = work.tile([128, DC, SP], F8, name=f"lcT8_{b}",
                                 tag="lcT8", bufs=NB)
                for c in range(DC):
                    nc.vector.tensor_scalar(out=lcT8[:, c, :],
                                            in0=lcT[:, c, :], scalar1=1.0,
                                            scalar2=None, op0=AL.mult)
            else:
                lcT8 = lcT
            lc_t[b] = lcT
            lc8_t[b] = lcT8

        def phase_b(b, s):
            """w1 matmuls -> interT."""
            lcT8 = lc8_t[b]
            # ---- w1 -> gelu -> interT ----
            interT = work.tile([128, 8, SP], W2T, name=f"interT{b}",
                               tag="interT", bufs=NB)
            for hk in range(8):
                ps = psmm(f"ps_w1{b}{hk}")
                mmdr(ps, [(w1W8[0][:, :, hk * 128:(hk + 1) * 128], lcT8),
                          (w1W8[1][:, :, hk * 128:(hk + 1) * 128], seqT8[b])],
                     F8_W1)
                gelu_act(interT[:, hk, :], ps,
                         w1b_t[:, hk:hk + 1] if w1b_t is not None else None,
                         scale=w1_scale)
            inter_t[b] = interT

        def phase_c(b, s):
            """l1/r1/l2/r2 fills + scans."""
            _, _, _, _, aB, naB, _ = bc_next[b]
            baseT = base_t[b]
            axb = ax_tile(f"axb{b}", "axs")
            for c in range(DC):
                eng("fill_base").tensor_tensor(out=axb[:, c, 1:SP + 1], in0=aB,
                                               in1=baseT[:, c, :], op=AL.mult)
            l1T = work.tile([128, DC, SP], CVT, name=f"l1T{b}", tag="l1T", bufs=NB)
            r1T = work.tile([128, DC, SP], CVT, name=f"r1T{b}", tag="r1T", bufs=NB)
            for c in range(DC):
                scan_fwd(eng("scan_l1"), l1T[:, c, :], naB, axb[:, c])
                scan_bwd(eng("scan_r1"), r1T[:, c, :], naB, axb[:, c])
            ax2 = ax_tile(f"ax2{b}", "axs")
            for c in range(DC):
                eng("fill_l2").tensor_tensor(out=ax2[:, c, 1:SP + 1], in0=aB,
                                             in1=l1T[:, c, :], op=AL.mult)
            l2T = work.tile([128, DC, SP], CVT, name=f"l2T{b}", tag="l2T", bufs=NB)
            for c in range(DC):
                scan_fwd(eng("scan_l2"), l2T[:, c, :], naB, ax2[:, c])
            ax2b = ax_tile(f"ax2b{b}", "axs")
            for c in range(DC):
                eng("fill_r2").tensor_tensor(out=ax2b[:, c, 1:SP + 1], in0=aB,
                                             in1=r1T[:, c, :], op=AL.mult)
            r2T = work.tile([128, DC, SP], CVT, name=f"r2T{b}", tag="r2T", bufs=NB)
            for c in range(DC):
                scan_bwd(eng("scan_r2"), r2T[:, c, :], naB, ax2b[:, c])
            scan_t[b] = (l1T, r1T, l2T, r2T)

        def phase_dc(b, s):
            """conv -> gT -> tsc accumulate."""
            baseT = base_t[b]
            l1T, r1T, l2T, r2T = scan_t[b]
            # ---- conv -> gelu -> gT; tsc accumulate ----
            pieces = [(2, baseT), (1, l1T), (3, r1T), (0, l2T), (4, r2T)]
            for c in range(DC):
                ps = psmm(f"ps_cv{b}{c}")
                mmdr(ps, [(cvW8[w][:, :, c * 128:(c + 1) * 128], piece)
                          for (w, piece) in pieces], F8_CV)
                gT = work.tile([128, SP], F16, name=f"gT{b}{c}", tag="gT",
                               bufs=4)
                gelu_act(gT, ps,
                         convb_t[:, c:c + 1] if convb_t is not None else None,
                         scale=cv_scale)
                mm_stat(0, sc4[:, b, c, :], gT,
                        start=(b == 0 and c == 0), stop=(b == NB - 1 and c == 1),
                        tail_start=False, tail_stop=False, tail=False)

        def phase_dw(b, s):
            """w2 -> gates/parent -> comp -> LN stats."""
            lcT = lc_t[b]
            interT = inter_t[b]
            # ---- w2 -> gates/parent -> comp ----
            comp = work.tile([128, DC, SP], F16, name=f"comp{b}", tag="compT",
                             bufs=NB)
            parT = work.tile([128, DC, SP], F16, name=f"parT{b}", tag="gpar",
                             bufs=2)
            for g in [3, 0, 1, 2]:
                for c in range(DC):
                    cc = g * DC + c
                    ps = psmm(f"ps_w2{b}{cc}")
                    mmdr(ps, [(w2W8[p][:, :, cc * 128:(cc + 1) * 128],
                               interT[:, 2 * p:2 * p + 2, :]) for p in range(4)],
                         F8_W2)
                    bias = w2b_t[:, cc:cc + 1] if w2b_t is not None else 0.0
                    if g == 3:
                        nc.scalar.activation(out=parT[:, c, :], in_=ps,
                                             func=AF.Identity, bias=bias,
                                             scale=w2_scale)
                    else:
                        gate = work.tile([128, SP], F16, name=f"gate{b}",
                                         tag="gate", bufs=4)
                        nc.scalar.activation(out=gate, in_=ps, func=AF.Sigmoid,
                                             bias=bias, scale=w2_scale)
                        src = [lcT, seqT[b], parT][g]
                        if g == 0:
                            nc.vector.tensor_tensor(out=comp[:, c, :], in0=gate,
                                                    in1=src[:, c, :], op=AL.mult)
                        else:
                            gm = work.tile([128, SP], F16, name=f"gm{b}",
                                           tag="gmt", bufs=4)
                            eng("gm").tensor_tensor(out=gm, in0=gate,
                                                    in1=src[:, c, :], op=AL.mult)
                            nc.vector.tensor_tensor(out=comp[:, c, :],
                                                    in0=comp[:, c, :],
                                                    in1=gm, op=AL.add)
            comp_t[b] = comp

            # ---- LN stats of comp ----
            for c in range(DC):
                mm_stat(1, ob4[:, b, :], comp[:, c, :],
                        start=(b == 0 and c == 0), stop=(b == NB - 1 and c == 1),
                        tail_start=(b == 0 and c == 0), tail_stop=False)
            for c in range(DC):
                sq = work.tile([128, SP], F16, name=f"sq{b}{c}", tag="sq",
                               bufs=4)
                eng("sq").tensor_tensor(out=sq, in0=comp[:, c, :],
                                        in1=comp[:, c, :], op=AL.mult)
                mm_stat(2, ob4[:, b, :], sq,
                        start=(b == 0 and c == 0), stop=(b == NB - 1 and c == 1),
                        tail_start=False,
                        tail_stop=(b == NB - 1 and c == 1))

        def tail_tp(s):
            """tp/active rows; needs only the tsc stats (conv phase) -> runs
            concurrently with the w2 phase."""
            last = (s == n_steps - 1)
            if not last:
                asq = row4("asq")
                nc.vector.tensor_tensor(out=asq, in0=a4, in1=a4, op=AL.mult)

            # tp = selp * sigmoid(tsc): the reference's
            # tp = e^{t-mx}selp/(e^{t-mx}selp + e^{-mx} + EPS) equals this up
            # to O(EPS); scores are O(1) so no overflow concern.
            tsc = read_stat(0, "tsc", tail=False,
                            bias=scb_t[:, 0:1] if scb_t is not None else None)
            sig = row4("sig", dt=F16)
            nc.scalar.activation(out=sig, in_=tsc, func=AF.Sigmoid)
            tpp = work.tile([NB, SPP], F16, name="tpp", tag="rowP", bufs=3)
            nc.vector.memset(tpp[:, 0:SPP:SPP - 1], 0.0)
            tp = tpp[:, 1:SP + 1]
            nc.vector.tensor_tensor(out=tp, in0=sig, in1=selp4, op=AL.mult)
            tpm = row4("tpm", dt=F16)
            nc.vector.tensor_tensor(out=tpm, in0=tp, in1=mask4, op=AL.mult)
            rC = row4("rC", dt=F16)
            nc.vector.tensor_tensor(out=rC, in0=mask4, in1=tpm, op=AL.subtract)

            if last:
                return tpm, rC, None, None, None

            # active update: a_new = clip(a - a^2*u, 0, 1)*mask
            nap = work.tile([NB, SPP], F16, name="nap", tag="rowP", bufs=3)
            nc.vector.memset(nap[:, 0:SPP:SPP - 1], 0.0)
            nc.vector.tensor_scalar(out=nap[:, 1:SP + 1], in0=a4,
                                    scalar1=-1.0, scalar2=1.0,
                                    op0=AL.mult, op1=AL.add)
            u = row4("u")
            nc.vector.tensor_tensor_scan(
                out=u[:, ::-1], data0=nap[:, SPP - 1:1:-1],
                data1=tpp[:, SPP - 1:1:-1], initial=0.0,
                op0=AL.mult, op1=AL.add)
            nd = row4("nd")
            nc.vector.tensor_tensor(out=nd, in0=asq, in1=u, op=AL.mult)
            nc.vector.tensor_tensor(out=nd, in0=a4, in1=nd, op=AL.subtract)
            nc.vector.tensor_scalar(out=nd, in0=nd, scalar1=0.0,
                                    scalar2=1.0, op0=AL.max, op1=AL.min)
            nc.vector.tensor_tensor(out=a4, in0=nd, in1=mask4, op=AL.mult)
            a_f = row4("a_f", dt=F16)
            nc.vector.tensor_scalar(out=a_f, in0=a4, scalar1=1.0,
                                    scalar2=None, op0=AL.mult)
            na_f = row4("na_f", dt=F16)
            nc.vector.tensor_scalar(out=na_f, in0=a4, scalar1=-1.0,
                                    scalar2=1.0, op0=AL.mult, op1=AL.add)
            return tpm, rC, a_f, na_f, tp

        def tail_bc(s, rows):
            """row broadcasts at the step boundary: PE is idle there and the
            rows were computed during the w2 phase.  Also precompute the
            apply's rC*seq_old terms here (Pool is idle; only the rA/rB part
            then waits on the LN chain)."""
            tpm, rC, a_f, na_f, tp = rows
            for b in range(NB):
                rCB = pe_bcast(rC, b, f"rCBs{s}{b}", "rCBt", "act")
                tpmB = (pe_bcast(tpm, b, f"tpmBs{s}{b}", "tpmBt", "act")
                        if lng_t is not None else None)
                t2s = []
                for c in range(DC):
                    t2 = work.tile([128, SP], F16, name=f"t2e{b}{c}",
                                   tag="t2e", bufs=2 * NB)
                    eng("t2").tensor_tensor(out=t2, in0=rCB,
                                            in1=seqT[b][:, c, :], op=AL.mult)
                    t2s.append(t2)
                t2_pre[b] = t2s
                if a_f is None:
                    bc_next[b] = (rCB, tpmB)
                else:
                    aB = pe_bcast(a_f, b, f"aBs{s}{b}", "aBt", "act")
                    naB = pe_bcast(na_f, b, f"naBs{s}{b}", None, "dve",
                                   pads=True)
                    ltpB = pe_bcast(tp, b, f"ltpBs{s}{b}", "ltpBt", "dve")
                    bc_next[b] = (rCB, tpmB, aB, naB, ltpB)

        def tail_ln(s, tpm):
            """LN gating rows; needs the w2-phase mean/var stats."""
            rstd, m_r = ln_rows()
            rA = row4("rA", dt=F16)
            nc.vector.tensor_tensor(out=rA, in0=tpm, in1=rstd, op=AL.mult)
            rB = row4("rB", dt=F16)
            nc.vector.tensor_tensor(out=rB, in0=rA, in1=m_r, op=AL.mult)
            for b in range(NB):
                rAB = pe_bcast(rA, b, f"rABs{s}{b}", "rABt", "dve")
                rBB = pe_bcast(rB, b, f"rBBs{s}{b}", "rBBt", "act")
                bc_next[b] = (rAB, rBB) + bc_next[b]

        for s in range(n_steps):
            for b in range(NB):
                phase_a(b, s)
            for b in range(NB):
                phase_b(b, s)
            for b in range(NB):
                phase_c(b, s)
            for b in range(NB):
                phase_dc(b, s)
            rows = tail_tp(s)
            for b in range(NB):
                phase_dw(b, s)
            tail_bc(s, rows)
            tail_ln(s, rows[0])

        # ---------------- final apply (f32 out) + DMA ----------------------
        outF = [work.tile([128, DC, SP], F32, name=f"outF{b}", tag="outF",
                          bufs=2) for b in range(NB)]

        def out_dma(b, c, dst):
            nc.sync.dma_start(out=out_dram.ap()[b, c], in_=dst[:, c, 0:S2])

        for b in range(NB):
            apply_ln(b, comp_t[b], bc_next[b][:4], outF, False,
                     per_chunk_dma=out_dma, t2_pre=t2_pre.get(b))
    return nc


def _host_prep(inputs):
    f32 = np.float32
    f16 = np.float16
    f8 = ml_dtypes.float8_e4m3
    seq = np.asarray(inputs["sequence"], f32)
    im = np.asarray(inputs["input_mask"], f32)
    START = np.asarray(inputs["START"], f32)
    END = np.asarray(inputs["END"], f32)
    yes_t = np.asarray(inputs["yes_t"], f32).reshape(-1)
    no_t = np.asarray(inputs["no_t"], f32).reshape(-1)
    N, S, Dd = seq.shape
    assert (N, S, Dd) == (32, 512, 256), (N, S, Dd)

    ones = np.ones((N, 1, 1), f32)
    zeros = np.zeros((N, 1, 1), f32)
    mask0 = np.concatenate([ones, im], 1)
    mask_no_end = np.concatenate([mask0, zeros], 1)
    mask_yes_end = np.concatenate([ones, mask0], 1)
    END_mask = mask_yes_end - mask_no_end
    seqA = np.concatenate([np.broadcast_to(START, (N, 1, Dd)), seq,
                           np.zeros((N, 1, Dd), f32)], 1)
    seqA = (END_mask * END + (1.0 - END_mask) * seqA).astype(f32)
    mask = mask_yes_end
    mask_no_start = np.concatenate([zeros, mask[:, 1:]], 1)
    last_tok = np.concatenate([END_mask[:, 1:], zeros], 1)
    selp = (mask_no_start * mask_no_end * (1.0 - last_tok)).astype(f32)

    seqT = np.zeros((N, DC, 128, SP), f32)
    for c in range(DC):
        seqT[:, c, :, :S2] = seqA[:, :, c * 128:(c + 1) * 128].transpose(0, 2, 1)
    maskP = np.zeros((N, SP), f32)
    maskP[:, :S2] = mask[:, :, 0]
    selpP = np.zeros((N, SP), f32)
    selpP[:, :S2] = selp[:, :, 0]
    actP = maskP.copy()
    nactP = (1.0 - actP).astype(f32)

    def chunk_col(v, nch):
        return np.ascontiguousarray(np.asarray(v, f32).reshape(nch, 128).T)

    scW = np.asarray(inputs["scW"], f32).reshape(-1)
    sc4 = np.zeros((128, NB, DC, 4), f32)
    for b in range(NB):
        for c in range(DC):
            sc4[:, b, c, b] = scW[c * 128:(c + 1) * 128]
    ob4 = np.zeros((128, NB, 4), f32)
    for b in range(NB):
        ob4[:, b, b] = 1.0
    bsel = np.zeros((4, NB, 128), f32)
    for b in range(NB):
        bsel[b, b, :] = 1.0

    def wconv(name, use_f8):
        w = np.asarray(inputs[name], f32)
        return (w * WSCALE).astype(f8) if use_f8 else w.astype(f16)

    host = {
        "seqT": seqT.astype(f16),
        "mask": maskP.astype(f16), "selp": selpP.astype(f16),
        "act0": actP.astype(f16), "act0f": actP,
        "nact0": nactP.astype(f16),
        "itW": np.asarray(inputs["itW"], f32).astype(f16),
        "convW": wconv("convW", F8_CV),
        "w1W": wconv("w1W", F8_W1),
        "w2W": wconv("w2W", F8_W2),
        "sc4": sc4.astype(f16),
        "ob4": ob4.astype(f16),
        "bsel": bsel.astype(f16),
        "noc": chunk_col(no_t, DC),
        "ymnc": chunk_col(yes_t - no_t, DC),
    }
    flags = {
        "itbc": bool(np.any(np.asarray(inputs["itb"]))),
        "convbc": bool(np.any(np.asarray(inputs["convb"]))),
        "w1bc": bool(np.any(np.asarray(inputs["w1b"]))),
        "w2bc": bool(np.any(np.asarray(inputs["w2b"]))),
        "scbc": bool(np.any(np.asarray(inputs["scb"]))),
        "lngc": bool(np.any(np.asarray(inputs["lnb"])))
        or bool(np.any(np.asarray(inputs["lng"]) != 1.0)),
    }
    flags["lnbc"] = flags["lngc"]
    if flags["itbc"]:
        host["itbc"] = chunk_col(inputs["itb"], DC)
    if flags["convbc"]:
        host["convbc"] = chunk_col(inputs["convb"], DC)
    if flags["w1bc"]:
        host["w1bc"] = chunk_col(inputs["w1b"], 8)
    if flags["w2bc"]:
        host["w2bc"] = chunk_col(inputs["w2b"], 8)
    if flags["scbc"]:
        host["scbc"] = np.broadcast_to(
            np.asarray(inputs["scb"], f32).reshape(1, 1), (4, 1)).copy()
    if flags["lngc"]:
        host["lngc"] = chunk_col(inputs["lng"], DC)
        host["lnbc"] = chunk_col(inputs["lnb"], DC)
    return host, flags


_PROG_CACHE = {}


def kernel(**inputs):
    global LAST_EXEC_NS, LAST_RES
    n_steps = int(inputs["n_steps"])
    host, flags = _host_prep(inputs)

    key = (n_steps, tuple(sorted(flags.items())), MM_DT, W2_DT, SIM, GP_LVL,
           F8_W1, F8_W2, F8_CV)
    if key not in _PROG_CACHE:
        _PROG_CACHE[key] = _build_program(n_steps, flags)
    nc = _PROG_CACHE[key]

    per_batch = {"seqT", "mask", "selp", "act0", "act0f", "nact0"}
    in_maps = []
    for k in range(NCORES):
        m = {}
        for name, arr in host.items():
            if name in per_batch:
                m[name] = np.ascontiguousarray(arr[k * NB:(k + 1) * NB])
            else:
                m[name] = arr
        in_maps.append(m)

    if SIM:
        from concourse.bass_interp import CoreSim
        results = []
        for k in range(int(os.environ.get("CRVNN_SIM_CORES", "1"))):
            sim = CoreSim(nc)
            for name, v in in_maps[k].items():
                sim.tensor(name)[:] = v
            sim.simulate()
            results.append(np.array(sim.tensor("out")))
    else:
        from concourse.bass_utils import run_bass_kernel_spmd
        if not getattr(nc, "_waitfix_done", False):
            _split_multiwaits(nc)
            nc._waitfix_done = True
        res = run_bass_kernel_spmd(nc, in_maps, list(range(NCORES)), trace=TRACE)
        LAST_EXEC_NS = res.exec_time_ns
        LAST_RES = res
        results = [res.results[k]["out"] for k in range(NCORES)]

    full = np.zeros((32, S2, D), np.float32)
    for k, o in enumerate(results):
        for b in range(NB):
            for c in range(DC):
                full[k * NB + b, :, c * 128:(c + 1) * 128] = \
                    np.asarray(o[b, c], np.float32).T
    return full
